# revision 20
# baseline (speedup 1.0000x reference)
"""NVFP4 fake-quantized linear layer on 8 Trainium2 NeuronCores.

Computes: y = x @ dequant(nvfp4_quantize(weight)).T + bias
  x [8192, 4096] f32, weight [4096, 4096] f32, bias [4096] f32.

Strategy (tensor-parallel, row-wise weight sharding, 512 rows/core):
  - x is transposed and cast to bf16 on the host (layout/precision prep);
    every core receives the full xT [K, M] so the matmul phase streams
    natural [128k, m] tiles with plain DMA - no transpose engine, no
    AllGather on the critical path.
  - Quantization (per-(row, 32-block) MSE scale search, bit-faithful fp32
    incl. fp8-e4m3 scale rounding emulated in fp32) is software-pipelined
    across engines with zero-stall buffer rotation:
      ScalarE (ratio i+1): Au = Relu(12 - c*a2s); Ac = 12 - Au  (clip)
      DVE     (ratio i):   tta = Ac & EXP_MASK (exp2 floor bits)
                           msv = max(tta*MAGIC/2, MAGIC)  (fused clamp:
                                 max(2^e,2)*M/2 == max(2^e*M/2, M))
                           r = Ac + msv ; q = r - msv  (magic RNE round)
                           d = a2s*c - q  (unclipped error -> wt buffer)
      ScalarE: dsq = Square(ratio*d)  (ratio^2 MSE weighting folded in)
      DVE:     e = per-32-block reduce(dsq); argmin bookkeeping
    TensorE transposes the dequantized weights into wdqT bf16.
  - Matmul runs over GROUP-PAIRS so each xt tile feeds 4 MMs (2 groups x
    2 m-groups), halving DMA bytes per matmul: pair {0,1} executes hidden
    under quantization of groups 2,3; pair {2,3} is the tail with inline
    per-chunk psum drains. Bias is added during the PSUM->SBUF drain; each
    core writes yT [512, 8192] and the host concatenates + transposes.
"""

import sys

sys.path.insert(0, "/opt/trn_rl_repo")

from contextlib import ExitStack

import numpy as np

import concourse.bass as bass
import concourse.bacc as bacc
import concourse.tile as tile
from concourse import mybir
from concourse.bass_utils import run_bass_kernel_spmd

A = mybir.AluOpType
AF = mybir.ActivationFunctionType
F32 = mybir.dt.float32
BF16 = mybir.dt.bfloat16
I32 = mybir.dt.int32

NCORES = 8
M, K, N = 8192, 4096, 4096
NSH = N // NCORES          # 512 weight rows per core
NG = NSH // 128            # 4 row groups per core
KC = K // 128              # 32 contraction chunks
KB = K // 32               # 128 blocks per weight row
MG = M // 512              # 16 output m-groups
MCH = 4                    # m-groups per psum chunk (4 banks live)

RATIOS = [float(r) for r in np.linspace(0.7, 1.0, 10)]
MAGIC = 12582912.0         # 1.5 * 2**23 : RNE-to-integer magic constant
INF = float("inf")
EXP_MASK = 0x7F800000      # fp32 exponent field mask
ABS_MASK = 0x7FFFFFFF      # clears the sign bit
TWO_BITS = 0x40000000      # bits of 2.0f; int max == float max for positives
# fp8 e4m3 rounding grid: step = max(2^-9, exp2floor(x) * 2^-3)
MAGIC8_HI = MAGIC / 8.0
MAGIC8_LO = MAGIC / 512.0


def build_nc() -> bass.Bass:
    nc = bacc.Bacc("TRN2", num_devices=NCORES)

    xT = nc.declare_dram_parameter("xT", [K, M], BF16, isOutput=False)
    w = nc.declare_dram_parameter("w", [NSH, K], F32, isOutput=False)
    bias = nc.declare_dram_parameter("bias", [NSH, 1], F32, isOutput=False)
    yT = nc.declare_dram_parameter("yT", [NSH, M], F32, isOutput=True)

    with tile.TileContext(nc) as tc, ExitStack() as ctx:
        big = ctx.enter_context(tc.tile_pool(name="big", bufs=1))
        sm = ctx.enter_context(tc.tile_pool(name="small", bufs=1))
        wtp = ctx.enter_context(tc.tile_pool(name="wtp", bufs=1))
        psum = ctx.enter_context(tc.tile_pool(name="psum", bufs=1, space="PSUM"))
        xtp = ctx.enter_context(tc.tile_pool(name="xtp", bufs=6))
        ytp = ctx.enter_context(tc.tile_pool(name="ytp", bufs=2))

        # persistent w_dq^T, bf16 [128 k-partitions, 32 k-chunks, 512 n]
        wdqT = big.tile([128, KC, NSH], BF16, tag="wdqT", name="wdqT")

        ident = sm.tile([128, 128], BF16, tag="ident", name="ident")
        from concourse.masks import make_identity

        make_identity(nc, ident)

        bias_sb = []
        for g in range(NG):
            bsl = sm.tile([128, 1], F32, tag=f"bias{g}", name=f"bias{g}")
            nc.scalar.dma_start(out=bsl, in_=bias[g * 128 : (g + 1) * 128, :])
            bias_sb.append(bsl)

        twelve = sm.tile([128, 1], F32, tag="twelve", name="twelve")
        nc.vector.memset(twelve, 12.0)
        magp = sm.tile([128, 1], F32, tag="magp", name="magp")
        nc.vector.memset(magp, MAGIC)
        magn = sm.tile([128, 1], F32, tag="magn", name="magn")
        nc.vector.memset(magn, -MAGIC)

        # hoisted per-ratio constant tiles for the argmin bookkeeping
        cconst, rconst = [], []
        for i, ratio in enumerate(RATIOS):
            ct = sm.tile([128, KB], F32, tag=f"cc{i}", name=f"cc{i}")
            nc.vector.memset(ct, float(np.float32(1.0) / np.float32(ratio)))
            cconst.append(ct)
            rt = sm.tile([128, KB], F32, tag=f"rc{i}", name=f"rc{i}")
            nc.vector.memset(rt, float(np.float32(ratio)))
            rconst.append(rt)

        def emit_quant_group(g):
            wt = wtp.tile([128, K], F32, tag="wt", name="wt")
            nc.scalar.dma_start(out=wt, in_=w[g * 128 : (g + 1) * 128, :])
            wt3 = wt.rearrange("p (b e) -> p b e", e=32)

            bmax = sm.tile([128, KB], F32, tag="bmax", name="bmax")
            nc.vector.tensor_reduce(
                out=bmax, in_=wt3, axis=mybir.AxisListType.X, op=A.max,
                apply_absolute_value=True,
            )
            nc.vector.tensor_scalar(out=bmax, in0=bmax, scalar1=1e-12, scalar2=None, op0=A.max)
            inv = sm.tile([128, KB], F32, tag="inv", name="inv")
            nc.vector.reciprocal(out=inv, in_=bmax)

            # b2s = w * 12 / bmax (signed);  a2s = |b2s| in [0, 12/ratio]
            b2s = big.tile([128, K], F32, tag="b2s", name="b2s")
            b2s3 = b2s.rearrange("p (b e) -> p b e", e=32)
            inv_b = inv.unsqueeze(2).broadcast_to([128, KB, 32])
            nc.vector.scalar_tensor_tensor(
                out=b2s3, in0=wt3, scalar=12.0, in1=inv_b, op0=A.mult, op1=A.mult,
            )
            a2s = big.tile([128, K], F32, tag="a2s", name="a2s")
            nc.scalar.activation(out=a2s, in_=b2s, func=AF.Abs)

            best_e = sm.tile([128, KB], F32, tag="best_e", name="best_e")
            nc.vector.memset(best_e, INF)
            best_c = sm.tile([128, KB], F32, tag="best_c", name="best_c")
            nc.vector.memset(best_c, 0.0)
            best_r = sm.tile([128, KB], F32, tag="best_r", name="best_r")
            nc.vector.memset(best_r, 1.0)

            # Software-pipelined MSE search: ScalarE computes ratio i+1's
            # clipped operand (Au/Ac) while DVE rounds ratio i; DVE hides
            # the Square latency under ratio i+1's tta/msv. Zero-stall.
            def emit_clip(i):
                # Ac = min(a2s*c_i, 12) via 12 - Relu(12 - c_i*a2s) on ScalarE
                c = float(np.float32(1.0) / np.float32(RATIOS[i]))
                Au = big.tile([128, K], F32, tag="sE", name="Au")
                nc.scalar.activation(
                    out=Au, in_=a2s, func=AF.Relu, scale=-c, bias=twelve,
                )
                Ac = big.tile([128, K], F32, tag="sA", name="Ac")
                nc.scalar.activation(
                    out=Ac, in_=Au, func=AF.Identity, scale=-1.0, bias=twelve,
                )
                return Ac

            def emit_mask(Ac):
                # tta = exp2floor-bits(Ac) on DVE (bitwise);
                # msv = max(tta*MAGIC/2, MAGIC) on ScalarE via
                # MAGIC + Relu(tta*MAGIC/2 - MAGIC) - exact: all terms are
                # 1.5*2^23*(2^j - 1) with tiny j, fp32-representable.
                tta = big.tile([128, K], F32, tag="sB", name="tta")
                nc.vector.tensor_scalar(
                    out=tta.bitcast(I32), in0=Ac.bitcast(I32),
                    scalar1=EXP_MASK, scalar2=None, op0=A.bitwise_and,
                )
                mu = big.tile([128, K], F32, tag="sC", name="mu")
                nc.scalar.activation(
                    out=mu, in_=tta, func=AF.Relu, scale=MAGIC / 2.0, bias=magn,
                )
                msv = big.tile([128, K], F32, tag="sD", name="msv")
                nc.scalar.activation(
                    out=msv, in_=mu, func=AF.Identity, scale=1.0, bias=magp,
                )
                return msv

            Ac = emit_clip(0)
            msv = emit_mask(Ac)
            for i, ratio in enumerate(RATIOS):
                c = float(np.float32(1.0) / np.float32(ratio))
                # r = Ac + msv ; q = r - msv  (RNE onto the e2m1 grid)
                r_ = big.tile([128, K], F32, tag="sC", name="r")
                nc.vector.tensor_tensor(out=r_, in0=Ac, in1=msv, op=A.add)
                q_ = big.tile([128, K], F32, tag="sB", name="q")
                nc.vector.scalar_tensor_tensor(
                    out=q_, in0=msv, scalar=-1.0, in1=r_, op0=A.mult, op1=A.add,
                )
                if i + 1 < len(RATIOS):
                    Ac = emit_clip(i + 1)
                # d = a2s*c - q (unclipped error, matches reference MSE);
                # lives in the idle wt buffer
                d_ = wtp.tile([128, K], F32, tag="wt", name="d")
                nc.vector.scalar_tensor_tensor(
                    out=d_, in0=a2s, scalar=c, in1=q_, op0=A.mult, op1=A.subtract,
                )
                if i + 1 < len(RATIOS):
                    msv = emit_mask(Ac)
                # dsq = (ratio*d)^2 : folds the ratio^2 MSE weighting in
                dsq = big.tile([128, K], F32, tag="sC", name="dsq")
                nc.scalar.activation(
                    out=dsq, in_=d_, func=AF.Square, scale=float(np.float32(ratio)),
                )
                e_ = sm.tile([128, KB], F32, tag="e", name="e")
                nc.vector.tensor_reduce(
                    out=e_, in_=dsq.rearrange("p (b e) -> p b e", e=32),
                    axis=mybir.AxisListType.X, op=A.add,
                )
                mask = sm.tile([128, KB], I32, tag="mask", name="mask")
                nc.vector.tensor_tensor(out=mask, in0=e_, in1=best_e, op=A.is_lt)
                nc.vector.tensor_tensor(out=best_e, in0=e_, in1=best_e, op=A.min)
                nc.vector.copy_predicated(out=best_c, mask=mask, data=cconst[i])
                nc.vector.copy_predicated(out=best_r, mask=mask, data=rconst[i])

            # scale factor sf = bmax * best_r / 6, rounded to fp8 e4m3 (RNE,
            # subnormal-aware) emulated in fp32, then halved (q = q2/2).
            sf = sm.tile([128, KB], F32, tag="sf", name="sf")
            nc.vector.scalar_tensor_tensor(
                out=sf, in0=bmax, scalar=1.0 / 6.0, in1=best_r, op0=A.mult, op1=A.mult,
            )
            eb8 = sm.tile([128, KB], F32, tag="eb8", name="eb8")
            nc.vector.tensor_scalar(
                out=eb8.bitcast(I32), in0=sf.bitcast(I32),
                scalar1=EXP_MASK, scalar2=None, op0=A.bitwise_and,
            )
            ms8 = sm.tile([128, KB], F32, tag="ms8", name="ms8")
            nc.vector.tensor_scalar(
                out=ms8, in0=eb8, scalar1=MAGIC8_HI, scalar2=MAGIC8_LO, op0=A.mult, op1=A.max,
            )
            nc.vector.tensor_tensor(out=sf, in0=sf, in1=ms8, op=A.add)
            nc.vector.tensor_tensor(out=sf, in0=sf, in1=ms8, op=A.subtract)
            nc.vector.tensor_scalar(out=sf, in0=sf, scalar1=0.5, scalar2=None, op0=A.mult)

            # final quantization with the chosen scale (signed)
            B2f = big.tile([128, K], F32, tag="sA", name="B2f")
            B2f3 = B2f.rearrange("p (b e) -> p b e", e=32)
            bc_b = best_c.unsqueeze(2).broadcast_to([128, KB, 32])
            nc.vector.tensor_tensor(out=B2f3, in0=b2s3, in1=bc_b, op=A.mult)
            ttaf = big.tile([128, K], F32, tag="sB", name="ttaf")
            nc.vector.tensor_scalar(
                out=ttaf.bitcast(I32), in0=B2f.bitcast(I32),
                scalar1=EXP_MASK, scalar2=None, op0=A.bitwise_and,
            )
            msvf = big.tile([128, K], F32, tag="sD", name="msvf")
            nc.vector.tensor_scalar(
                out=msvf, in0=ttaf, scalar1=MAGIC / 2.0, scalar2=MAGIC,
                op0=A.mult, op1=A.max,
            )
            rf = big.tile([128, K], F32, tag="sC", name="rf")
            nc.vector.tensor_tensor(out=rf, in0=B2f, in1=msvf, op=A.add)
            qf = big.tile([128, K], F32, tag="sA", name="qf")
            nc.vector.scalar_tensor_tensor(
                out=qf, in0=msvf, scalar=-1.0, in1=rf, op0=A.mult, op1=A.add,
            )
            qc = big.tile([128, K], F32, tag="sB", name="qc")
            nc.vector.tensor_scalar(
                out=qc, in0=qf, scalar1=12.0, scalar2=-12.0, op0=A.min, op1=A.max,
            )
            wdq = big.tile([128, K], BF16, tag="wdq", name="wdq")
            sf_b = sf.unsqueeze(2).broadcast_to([128, KB, 32])
            nc.vector.tensor_tensor(
                out=wdq.rearrange("p (b e) -> p b e", e=32),
                in0=qc.rearrange("p (b e) -> p b e", e=32),
                in1=sf_b, op=A.mult,
            )

            # transpose into wdqT[:, kc, g*128:(g+1)*128]
            for kc in range(KC):
                pt = psum.tile([128, 128], BF16, tag="ptr", bufs=2, name="pt")
                nc.tensor.transpose(pt, wdq[:, kc * 128 : (kc + 1) * 128], ident)
                nc.scalar.copy(out=wdqT[:, kc, g * 128 : (g + 1) * 128], in_=pt)

        # Matmul runs over GROUP-PAIRS: each xt tile feeds 4 MMs (2 groups
        # x 2 m-groups), halving DMA bytes per MM vs per-group passes.
        # Pair {0,1} hides under quant of groups 2,3; pair {2,3} is the tail.
        pair_psums = {}

        def emit_pair_mms(pair, rings, inline_drain=False):
            g0 = 2 * pair
            pair_psums[pair] = []
            for mc in range(M // 1024):
                psums = [
                    psum.tile([128, 512], F32, tag=f"pp{j}", name=f"pp{j}")
                    for j in range(4)
                ]
                pair_psums[pair].append(psums)
                for kc in range(KC):
                    xt = xtp.tile([128, 1024], BF16, tag="xt", name="xt")
                    rings[(mc * KC + kc) % len(rings)].dma_start(
                        out=xt,
                        in_=xT[kc * 128 : (kc + 1) * 128,
                               mc * 1024 : (mc + 1) * 1024],
                    )
                    for gi in range(2):
                        for j in range(2):
                            nc.tensor.matmul(
                                psums[gi * 2 + j],
                                lhsT=wdqT[:, kc, (g0 + gi) * 128 : (g0 + gi + 1) * 128],
                                rhs=xt[:, j * 512 : (j + 1) * 512],
                                start=(kc == 0),
                                stop=(kc == KC - 1),
                            )
                if inline_drain:
                    emit_chunk_drain(g0, mc, psums)

        def emit_chunk_drain(g0, mc, psums):
            for gi in range(2):
                for j in range(2):
                    ysb = ytp.tile([128, 512], F32, tag="ysb", name="ysb")
                    nc.scalar.add(out=ysb, in_=psums[gi * 2 + j], add=bias_sb[g0 + gi])
                    g, mg = g0 + gi, mc * 2 + j
                    nc.sync.dma_start(
                        out=yT[g * 128 : (g + 1) * 128, mg * 512 : (mg + 1) * 512],
                        in_=ysb,
                    )

        def emit_pair_tail(pair):
            # bias-add drain on ScalarE - only ever emitted after all quant
            # scalar work, so it cannot stall the quant chain.
            g0 = 2 * pair
            for mc, psums in enumerate(pair_psums[pair]):
                for gi in range(2):
                    for j in range(2):
                        ysb = ytp.tile([128, 512], F32, tag="ysb", name="ysb")
                        nc.scalar.add(out=ysb, in_=psums[gi * 2 + j], add=bias_sb[g0 + gi])
                        g, mg = g0 + gi, mc * 2 + j
                        nc.sync.dma_start(
                            out=yT[g * 128 : (g + 1) * 128, mg * 512 : (mg + 1) * 512],
                            in_=ysb,
                        )

        emit_quant_group(0)
        emit_quant_group(1)
        emit_pair_mms(0, [nc.sync, nc.gpsimd])
        emit_quant_group(2)
        emit_quant_group(3)
        emit_pair_tail(0)
        emit_pair_mms(1, [nc.sync, nc.gpsimd], inline_drain=True)

    nc.compile()
    return nc


_NC_CACHE = None


def _in_maps(x, weight, bias):
    import ml_dtypes

    x = np.ascontiguousarray(x, dtype=np.float32)
    weight = np.ascontiguousarray(weight, dtype=np.float32)
    bias = np.ascontiguousarray(bias, dtype=np.float32)
    xT = np.ascontiguousarray(x.T).astype(ml_dtypes.bfloat16)
    in_maps = []
    for c in range(NCORES):
        in_maps.append(
            {
                "xT": xT,
                "w": weight[c * NSH : (c + 1) * NSH],
                "bias": bias[c * NSH : (c + 1) * NSH].reshape(NSH, 1),
            }
        )
    return in_maps


def kernel(x: np.ndarray, weight: np.ndarray, bias: np.ndarray) -> np.ndarray:
    global _NC_CACHE
    if _NC_CACHE is None:
        _NC_CACHE = build_nc()
    nc = _NC_CACHE
    res = run_bass_kernel_spmd(nc, _in_maps(x, weight, bias), list(range(NCORES)))
    yT = np.concatenate([res.results[c]["yT"] for c in range(NCORES)], axis=0)
    return np.ascontiguousarray(yT.T)


def profile_once(x, weight, bias):
    global _NC_CACHE
    if _NC_CACHE is None:
        _NC_CACHE = build_nc()
    nc = _NC_CACHE
    res = run_bass_kernel_spmd(
        nc, _in_maps(x, weight, bias), list(range(NCORES)),
        trace=True, tmpdir="/tmp/nvfp4_trace",
    )
    print("exec_time_ns:", res.exec_time_ns, "mean:", res.mean_exec_time_ns,
          "max_core:", res.max_exec_time_core_id)
    return res.exec_time_ns


# revision 21
# speedup vs baseline: 1.4422x; 1.4422x over previous
"""NVFP4 fake-quantized linear layer on 8 Trainium2 NeuronCores.

Computes: y = x @ dequant(nvfp4_quantize(weight)).T + bias
  x [8192, 4096] f32, weight [4096, 4096] f32, bias [4096] f32.

Strategy (tensor-parallel, row-wise weight sharding, 512 rows/core):
  - x is transposed and cast to bf16 on the host (layout/precision prep);
    every core receives the full xT [K, M] so the matmul phase streams
    natural [128k, m] tiles with plain DMA - no transpose engine, no
    AllGather on the critical path.
  - Quantization (per-(row, 32-block) MSE scale search, bit-faithful fp32
    incl. fp8-e4m3 scale rounding emulated in fp32) is software-pipelined
    across engines with zero-stall buffer rotation:
      ScalarE (ratio i+1): Au = Relu(12 - c*a2s); Ac = 12 - Au  (clip)
      DVE     (ratio i):   tta = Ac & EXP_MASK (exp2 floor bits)
                           msv = max(tta*MAGIC/2, MAGIC)  (fused clamp:
                                 max(2^e,2)*M/2 == max(2^e*M/2, M))
                           r = Ac + msv ; q = r - msv  (magic RNE round)
                           d = a2s*c - q  (unclipped error -> wt buffer)
      ScalarE: dsq = Square(ratio*d)  (ratio^2 MSE weighting folded in)
      DVE:     e = per-32-block reduce(dsq); argmin bookkeeping
    TensorE transposes the dequantized weights into wdqT bf16.
  - Matmul runs over GROUP-PAIRS so each xt tile feeds 4 MMs (2 groups x
    2 m-groups), halving DMA bytes per matmul: pair {0,1} executes hidden
    under quantization of groups 2,3; pair {2,3} is the tail with inline
    per-chunk psum drains. Bias is added during the PSUM->SBUF drain; each
    core writes yT [512, 8192] and the host concatenates + transposes.
"""

import sys

sys.path.insert(0, "/opt/trn_rl_repo")

from contextlib import ExitStack

import numpy as np

import concourse.bass as bass
import concourse.bacc as bacc
import concourse.tile as tile
from concourse import mybir
from concourse.bass_utils import run_bass_kernel_spmd

A = mybir.AluOpType
AF = mybir.ActivationFunctionType
F32 = mybir.dt.float32
BF16 = mybir.dt.bfloat16
I32 = mybir.dt.int32

NCORES = 8
M, K, N = 8192, 4096, 4096
NSH = N // NCORES          # 512 weight rows per core
NG = NSH // 128            # 4 row groups per core
KC = K // 128              # 32 contraction chunks
KB = K // 32               # 128 blocks per weight row
MG = M // 512              # 16 output m-groups
MCH = 4                    # m-groups per psum chunk (4 banks live)

RATIOS = [float(r) for r in np.linspace(0.7, 1.0, 10)]
MAGIC = 12582912.0         # 1.5 * 2**23 : RNE-to-integer magic constant
INF = float("inf")
EXP_MASK = 0x7F800000      # fp32 exponent field mask
ABS_MASK = 0x7FFFFFFF      # clears the sign bit
TWO_BITS = 0x40000000      # bits of 2.0f; int max == float max for positives
# fp8 e4m3 rounding grid: step = max(2^-9, exp2floor(x) * 2^-3)
MAGIC8_HI = MAGIC / 8.0
MAGIC8_LO = MAGIC / 512.0


def build_nc() -> bass.Bass:
    nc = bacc.Bacc("TRN2", num_devices=NCORES)

    xT = nc.declare_dram_parameter("xT", [K, M], BF16, isOutput=False)
    w = nc.declare_dram_parameter("w", [NSH, K], F32, isOutput=False)
    bias = nc.declare_dram_parameter("bias", [NSH, 1], F32, isOutput=False)
    yT = nc.declare_dram_parameter("yT", [NSH, M], F32, isOutput=True)

    with tile.TileContext(nc) as tc, ExitStack() as ctx:
        big = ctx.enter_context(tc.tile_pool(name="big", bufs=1))
        sm = ctx.enter_context(tc.tile_pool(name="small", bufs=1))
        wtp = ctx.enter_context(tc.tile_pool(name="wtp", bufs=1))
        psum = ctx.enter_context(tc.tile_pool(name="psum", bufs=1, space="PSUM"))
        xtp = ctx.enter_context(tc.tile_pool(name="xtp", bufs=6))
        ytp = ctx.enter_context(tc.tile_pool(name="ytp", bufs=2))

        # persistent w_dq^T, bf16 [128 k-partitions, 32 k-chunks, 512 n]
        wdqT = big.tile([128, KC, NSH], BF16, tag="wdqT", name="wdqT")

        ident = sm.tile([128, 128], BF16, tag="ident", name="ident")
        from concourse.masks import make_identity

        make_identity(nc, ident)

        bias_sb = []
        for g in range(NG):
            bsl = sm.tile([128, 1], F32, tag=f"bias{g}", name=f"bias{g}")
            nc.scalar.dma_start(out=bsl, in_=bias[g * 128 : (g + 1) * 128, :])
            bias_sb.append(bsl)

        twelve = sm.tile([128, 1], F32, tag="twelve", name="twelve")
        nc.vector.memset(twelve, 12.0)

        # hoisted per-ratio constant tiles for the argmin bookkeeping
        cconst, rconst = [], []
        for i, ratio in enumerate(RATIOS):
            ct = sm.tile([128, KB], F32, tag=f"cc{i}", name=f"cc{i}")
            nc.vector.memset(ct, float(np.float32(1.0) / np.float32(ratio)))
            cconst.append(ct)
            rt = sm.tile([128, KB], F32, tag=f"rc{i}", name=f"rc{i}")
            nc.vector.memset(rt, float(np.float32(ratio)))
            rconst.append(rt)

        def emit_quant_group(g):
            wt = wtp.tile([128, K], F32, tag="wt", name="wt")
            nc.scalar.dma_start(out=wt, in_=w[g * 128 : (g + 1) * 128, :])
            wt3 = wt.rearrange("p (b e) -> p b e", e=32)

            bmax = sm.tile([128, KB], F32, tag="bmax", name="bmax")
            nc.vector.tensor_reduce(
                out=bmax, in_=wt3, axis=mybir.AxisListType.X, op=A.max,
                apply_absolute_value=True,
            )
            nc.vector.tensor_scalar(out=bmax, in0=bmax, scalar1=1e-12, scalar2=None, op0=A.max)
            inv = sm.tile([128, KB], F32, tag="inv", name="inv")
            nc.vector.reciprocal(out=inv, in_=bmax)

            # b2s = w * 12 / bmax (signed);  a2s = |b2s| in [0, 12/ratio]
            b2s = big.tile([128, K], F32, tag="b2s", name="b2s")
            b2s3 = b2s.rearrange("p (b e) -> p b e", e=32)
            inv_b = inv.unsqueeze(2).broadcast_to([128, KB, 32])
            nc.vector.scalar_tensor_tensor(
                out=b2s3, in0=wt3, scalar=12.0, in1=inv_b, op0=A.mult, op1=A.mult,
            )
            a2s = big.tile([128, K], F32, tag="a2s", name="a2s")
            nc.scalar.activation(out=a2s, in_=b2s, func=AF.Abs)

            best_e = sm.tile([128, KB], F32, tag="best_e", name="best_e")
            nc.vector.memset(best_e, INF)
            best_c = sm.tile([128, KB], F32, tag="best_c", name="best_c")
            nc.vector.memset(best_c, 0.0)
            best_r = sm.tile([128, KB], F32, tag="best_r", name="best_r")
            nc.vector.memset(best_r, 1.0)

            # Software-pipelined MSE search: ScalarE computes ratio i+1's
            # clipped operand (Au/Ac) while DVE rounds ratio i; DVE hides
            # the Square latency under ratio i+1's tta/msv. Zero-stall.
            def emit_clip(i):
                # Ac = min(a2s*c_i, 12) via 12 - Relu(12 - c_i*a2s) on ScalarE
                c = float(np.float32(1.0) / np.float32(RATIOS[i]))
                Au = big.tile([128, K], F32, tag="sE", name="Au")
                nc.scalar.activation(
                    out=Au, in_=a2s, func=AF.Relu, scale=-c, bias=twelve,
                )
                Ac = big.tile([128, K], F32, tag="sA", name="Ac")
                nc.scalar.activation(
                    out=Ac, in_=Au, func=AF.Identity, scale=-1.0, bias=twelve,
                )
                return Ac

            def emit_mask(Ac):
                # tta = exp2floor-bits(Ac);  msv = max(tta*MAGIC/2, MAGIC)
                tta = big.tile([128, K], F32, tag="sB", name="tta")
                nc.vector.tensor_scalar(
                    out=tta.bitcast(I32), in0=Ac.bitcast(I32),
                    scalar1=EXP_MASK, scalar2=None, op0=A.bitwise_and,
                )
                msv = big.tile([128, K], F32, tag="sD", name="msv")
                nc.vector.tensor_scalar(
                    out=msv, in0=tta, scalar1=MAGIC / 2.0, scalar2=MAGIC,
                    op0=A.mult, op1=A.max,
                )
                return msv

            Ac = emit_clip(0)
            msv = emit_mask(Ac)
            for i, ratio in enumerate(RATIOS):
                c = float(np.float32(1.0) / np.float32(ratio))
                # r = Ac + msv ; q = r - msv  (RNE onto the e2m1 grid)
                r_ = big.tile([128, K], F32, tag="sC", name="r")
                nc.vector.tensor_tensor(out=r_, in0=Ac, in1=msv, op=A.add)
                q_ = big.tile([128, K], F32, tag="sB", name="q")
                nc.vector.scalar_tensor_tensor(
                    out=q_, in0=msv, scalar=-1.0, in1=r_, op0=A.mult, op1=A.add,
                )
                if i + 1 < len(RATIOS):
                    Ac = emit_clip(i + 1)
                # d = a2s*c - q (unclipped error, matches reference MSE);
                # lives in the idle wt buffer
                d_ = wtp.tile([128, K], F32, tag="wt", name="d")
                nc.vector.scalar_tensor_tensor(
                    out=d_, in0=a2s, scalar=c, in1=q_, op0=A.mult, op1=A.subtract,
                )
                if i + 1 < len(RATIOS):
                    msv = emit_mask(Ac)
                # dsq = (ratio*d)^2 : folds the ratio^2 MSE weighting in
                dsq = big.tile([128, K], F32, tag="sC", name="dsq")
                nc.scalar.activation(
                    out=dsq, in_=d_, func=AF.Square, scale=float(np.float32(ratio)),
                )
                e_ = sm.tile([128, KB], F32, tag="e", name="e")
                nc.vector.tensor_reduce(
                    out=e_, in_=dsq.rearrange("p (b e) -> p b e", e=32),
                    axis=mybir.AxisListType.X, op=A.add,
                )
                mask = sm.tile([128, KB], I32, tag="mask", name="mask")
                nc.vector.tensor_tensor(out=mask, in0=e_, in1=best_e, op=A.is_lt)
                nc.vector.tensor_tensor(out=best_e, in0=e_, in1=best_e, op=A.min)
                nc.vector.copy_predicated(out=best_c, mask=mask, data=cconst[i])
                nc.vector.copy_predicated(out=best_r, mask=mask, data=rconst[i])

            # scale factor sf = bmax * best_r / 6, rounded to fp8 e4m3 (RNE,
            # subnormal-aware) emulated in fp32, then halved (q = q2/2).
            sf = sm.tile([128, KB], F32, tag="sf", name="sf")
            nc.vector.scalar_tensor_tensor(
                out=sf, in0=bmax, scalar=1.0 / 6.0, in1=best_r, op0=A.mult, op1=A.mult,
            )
            eb8 = sm.tile([128, KB], F32, tag="eb8", name="eb8")
            nc.vector.tensor_scalar(
                out=eb8.bitcast(I32), in0=sf.bitcast(I32),
                scalar1=EXP_MASK, scalar2=None, op0=A.bitwise_and,
            )
            ms8 = sm.tile([128, KB], F32, tag="ms8", name="ms8")
            nc.vector.tensor_scalar(
                out=ms8, in0=eb8, scalar1=MAGIC8_HI, scalar2=MAGIC8_LO, op0=A.mult, op1=A.max,
            )
            nc.vector.tensor_tensor(out=sf, in0=sf, in1=ms8, op=A.add)
            nc.vector.tensor_tensor(out=sf, in0=sf, in1=ms8, op=A.subtract)
            nc.vector.tensor_scalar(out=sf, in0=sf, scalar1=0.5, scalar2=None, op0=A.mult)

            # final quantization with the chosen scale (signed)
            B2f = big.tile([128, K], F32, tag="sA", name="B2f")
            B2f3 = B2f.rearrange("p (b e) -> p b e", e=32)
            bc_b = best_c.unsqueeze(2).broadcast_to([128, KB, 32])
            nc.vector.tensor_tensor(out=B2f3, in0=b2s3, in1=bc_b, op=A.mult)
            ttaf = big.tile([128, K], F32, tag="sB", name="ttaf")
            nc.vector.tensor_scalar(
                out=ttaf.bitcast(I32), in0=B2f.bitcast(I32),
                scalar1=EXP_MASK, scalar2=None, op0=A.bitwise_and,
            )
            msvf = big.tile([128, K], F32, tag="sD", name="msvf")
            nc.vector.tensor_scalar(
                out=msvf, in0=ttaf, scalar1=MAGIC / 2.0, scalar2=MAGIC,
                op0=A.mult, op1=A.max,
            )
            rf = big.tile([128, K], F32, tag="sC", name="rf")
            nc.vector.tensor_tensor(out=rf, in0=B2f, in1=msvf, op=A.add)
            qf = big.tile([128, K], F32, tag="sA", name="qf")
            nc.vector.scalar_tensor_tensor(
                out=qf, in0=msvf, scalar=-1.0, in1=rf, op0=A.mult, op1=A.add,
            )
            qc = big.tile([128, K], F32, tag="sB", name="qc")
            nc.vector.tensor_scalar(
                out=qc, in0=qf, scalar1=12.0, scalar2=-12.0, op0=A.min, op1=A.max,
            )
            wdq = big.tile([128, K], BF16, tag="wdq", name="wdq")
            sf_b = sf.unsqueeze(2).broadcast_to([128, KB, 32])
            nc.vector.tensor_tensor(
                out=wdq.rearrange("p (b e) -> p b e", e=32),
                in0=qc.rearrange("p (b e) -> p b e", e=32),
                in1=sf_b, op=A.mult,
            )

            # transpose into wdqT[:, kc, g*128:(g+1)*128]
            for kc in range(KC):
                pt = psum.tile([128, 128], BF16, tag="ptr", bufs=2, name="pt")
                nc.tensor.transpose(pt, wdq[:, kc * 128 : (kc + 1) * 128], ident)
                nc.scalar.copy(out=wdqT[:, kc, g * 128 : (g + 1) * 128], in_=pt)

        # Matmul runs over GROUP-PAIRS: each xt tile feeds 4 MMs (2 groups
        # x 2 m-groups), halving DMA bytes per MM vs per-group passes.
        # Pair {0,1} hides under quant of groups 2,3; pair {2,3} is the tail.
        pair_psums = {}

        def emit_pair_mms(pair, rings, inline_drain=False):
            g0 = 2 * pair
            pair_psums[pair] = []
            for mc in range(M // 1024):
                psums = [
                    psum.tile([128, 512], F32, tag=f"pp{j}", name=f"pp{j}")
                    for j in range(4)
                ]
                pair_psums[pair].append(psums)
                for kc in range(KC):
                    xt = xtp.tile([128, 1024], BF16, tag="xt", name="xt")
                    rings[(mc * KC + kc) % len(rings)].dma_start(
                        out=xt,
                        in_=xT[kc * 128 : (kc + 1) * 128,
                               mc * 1024 : (mc + 1) * 1024],
                    )
                    for gi in range(2):
                        for j in range(2):
                            nc.tensor.matmul(
                                psums[gi * 2 + j],
                                lhsT=wdqT[:, kc, (g0 + gi) * 128 : (g0 + gi + 1) * 128],
                                rhs=xt[:, j * 512 : (j + 1) * 512],
                                start=(kc == 0),
                                stop=(kc == KC - 1),
                            )
                if inline_drain:
                    emit_chunk_drain(g0, mc, psums)

        def emit_chunk_drain(g0, mc, psums):
            for gi in range(2):
                for j in range(2):
                    ysb = ytp.tile([128, 512], F32, tag="ysb", name="ysb")
                    nc.scalar.add(out=ysb, in_=psums[gi * 2 + j], add=bias_sb[g0 + gi])
                    g, mg = g0 + gi, mc * 2 + j
                    nc.sync.dma_start(
                        out=yT[g * 128 : (g + 1) * 128, mg * 512 : (mg + 1) * 512],
                        in_=ysb,
                    )

        def emit_pair_tail(pair):
            # bias-add drain on ScalarE - only ever emitted after all quant
            # scalar work, so it cannot stall the quant chain.
            g0 = 2 * pair
            for mc, psums in enumerate(pair_psums[pair]):
                for gi in range(2):
                    for j in range(2):
                        ysb = ytp.tile([128, 512], F32, tag="ysb", name="ysb")
                        nc.scalar.add(out=ysb, in_=psums[gi * 2 + j], add=bias_sb[g0 + gi])
                        g, mg = g0 + gi, mc * 2 + j
                        nc.sync.dma_start(
                            out=yT[g * 128 : (g + 1) * 128, mg * 512 : (mg + 1) * 512],
                            in_=ysb,
                        )

        emit_quant_group(0)
        emit_quant_group(1)
        emit_pair_mms(0, [nc.sync, nc.gpsimd])
        emit_quant_group(2)
        emit_quant_group(3)
        emit_pair_tail(0)
        emit_pair_mms(1, [nc.sync, nc.gpsimd], inline_drain=True)

    nc.compile()
    return nc


_NC_CACHE = None


def _in_maps(x, weight, bias):
    import ml_dtypes

    x = np.ascontiguousarray(x, dtype=np.float32)
    weight = np.ascontiguousarray(weight, dtype=np.float32)
    bias = np.ascontiguousarray(bias, dtype=np.float32)
    xT = np.ascontiguousarray(x.T).astype(ml_dtypes.bfloat16)
    in_maps = []
    for c in range(NCORES):
        in_maps.append(
            {
                "xT": xT,
                "w": weight[c * NSH : (c + 1) * NSH],
                "bias": bias[c * NSH : (c + 1) * NSH].reshape(NSH, 1),
            }
        )
    return in_maps


def kernel(x: np.ndarray, weight: np.ndarray, bias: np.ndarray) -> np.ndarray:
    global _NC_CACHE
    if _NC_CACHE is None:
        _NC_CACHE = build_nc()
    nc = _NC_CACHE
    res = run_bass_kernel_spmd(nc, _in_maps(x, weight, bias), list(range(NCORES)))
    yT = np.concatenate([res.results[c]["yT"] for c in range(NCORES)], axis=0)
    return np.ascontiguousarray(yT.T)


def profile_once(x, weight, bias):
    global _NC_CACHE
    if _NC_CACHE is None:
        _NC_CACHE = build_nc()
    nc = _NC_CACHE
    res = run_bass_kernel_spmd(
        nc, _in_maps(x, weight, bias), list(range(NCORES)),
        trace=True, tmpdir="/tmp/nvfp4_trace",
    )
    print("exec_time_ns:", res.exec_time_ns, "mean:", res.mean_exec_time_ns,
          "max_core:", res.max_exec_time_core_id)
    return res.exec_time_ns


# revision 22
# speedup vs baseline: 1.4936x; 1.0356x over previous
"""NVFP4 fake-quantized linear layer on 8 Trainium2 NeuronCores.

Computes: y = x @ dequant(nvfp4_quantize(weight)).T + bias
  x [8192, 4096] f32, weight [4096, 4096] f32, bias [4096] f32.

Strategy (tensor-parallel, row-wise weight sharding, 512 rows/core):
  - x is transposed and cast to bf16 on the host (layout/precision prep);
    every core receives the full xT [K, M] so the matmul phase streams
    natural [128k, m] tiles with plain DMA - no transpose engine, no
    AllGather on the critical path.
  - Quantization (per-(row, 32-block) MSE scale search, bit-faithful fp32
    incl. fp8-e4m3 scale rounding emulated in fp32) is software-pipelined
    across engines with zero-stall buffer rotation:
      ScalarE (ratio i+1): Au = Relu(12 - c*a2s); Ac = 12 - Au  (clip)
      DVE     (ratio i):   tta = Ac & EXP_MASK (exp2 floor bits)
                           msv = max(tta*MAGIC/2, MAGIC)  (fused clamp:
                                 max(2^e,2)*M/2 == max(2^e*M/2, M))
                           r = Ac + msv ; q = r - msv  (magic RNE round)
                           d = a2s*c - q  (unclipped error -> wt buffer)
      ScalarE: dsq = Square(ratio*d)  (ratio^2 MSE weighting folded in)
      DVE:     e = per-32-block reduce(dsq); argmin bookkeeping
    TensorE transposes the dequantized weights into wdqT bf16.
  - Matmul runs over GROUP-PAIRS so each xt tile feeds 4 MMs (2 groups x
    2 m-groups), halving DMA bytes per matmul: pair {0,1} executes hidden
    under quantization of groups 2,3; pair {2,3} is the tail with inline
    per-chunk psum drains. Bias is added during the PSUM->SBUF drain; each
    core writes yT [512, 8192] and the host concatenates + transposes.
"""

import sys

sys.path.insert(0, "/opt/trn_rl_repo")

from contextlib import ExitStack

import numpy as np

import concourse.bass as bass
import concourse.bacc as bacc
import concourse.tile as tile
from concourse import mybir
from concourse.bass_utils import run_bass_kernel_spmd

A = mybir.AluOpType
AF = mybir.ActivationFunctionType
F32 = mybir.dt.float32
BF16 = mybir.dt.bfloat16
I32 = mybir.dt.int32

NCORES = 8
M, K, N = 8192, 4096, 4096
NSH = N // NCORES          # 512 weight rows per core
NG = NSH // 128            # 4 row groups per core
KC = K // 128              # 32 contraction chunks
KB = K // 32               # 128 blocks per weight row
MG = M // 512              # 16 output m-groups
MCH = 4                    # m-groups per psum chunk (4 banks live)

RATIOS = [float(r) for r in np.linspace(0.7, 1.0, 10)]
MAGIC = 12582912.0         # 1.5 * 2**23 : RNE-to-integer magic constant
INF = float("inf")
EXP_MASK = 0x7F800000      # fp32 exponent field mask
ABS_MASK = 0x7FFFFFFF      # clears the sign bit
TWO_BITS = 0x40000000      # bits of 2.0f; int max == float max for positives
# fp8 e4m3 rounding grid: step = max(2^-9, exp2floor(x) * 2^-3)
MAGIC8_HI = MAGIC / 8.0
MAGIC8_LO = MAGIC / 512.0


def build_nc() -> bass.Bass:
    nc = bacc.Bacc("TRN2", num_devices=NCORES)

    xT = nc.declare_dram_parameter("xT", [K, M], BF16, isOutput=False)
    w = nc.declare_dram_parameter("w", [NSH, K], F32, isOutput=False)
    bias = nc.declare_dram_parameter("bias", [NSH, 1], F32, isOutput=False)
    yT = nc.declare_dram_parameter("yT", [NSH, M], F32, isOutput=True)

    with tile.TileContext(nc) as tc, ExitStack() as ctx:
        big = ctx.enter_context(tc.tile_pool(name="big", bufs=1))
        sm = ctx.enter_context(tc.tile_pool(name="small", bufs=1))
        wtp = ctx.enter_context(tc.tile_pool(name="wtp", bufs=1))
        psum = ctx.enter_context(tc.tile_pool(name="psum", bufs=1, space="PSUM"))
        xtp = ctx.enter_context(tc.tile_pool(name="xtp", bufs=8))
        ytp = ctx.enter_context(tc.tile_pool(name="ytp", bufs=2))

        # persistent w_dq^T, bf16 [128 k-partitions, 32 k-chunks, 512 n]
        wdqT = big.tile([128, KC, NSH], BF16, tag="wdqT", name="wdqT")

        ident = sm.tile([128, 128], BF16, tag="ident", name="ident")
        from concourse.masks import make_identity

        make_identity(nc, ident)

        bias_sb = []
        for g in range(NG):
            bsl = sm.tile([128, 1], F32, tag=f"bias{g}", name=f"bias{g}")
            nc.scalar.dma_start(out=bsl, in_=bias[g * 128 : (g + 1) * 128, :])
            bias_sb.append(bsl)

        twelve = sm.tile([128, 1], F32, tag="twelve", name="twelve")
        nc.vector.memset(twelve, 12.0)

        # hoisted per-ratio constant tiles for the argmin bookkeeping
        cconst, rconst = [], []
        for i, ratio in enumerate(RATIOS):
            ct = sm.tile([128, KB], F32, tag=f"cc{i}", name=f"cc{i}")
            nc.vector.memset(ct, float(np.float32(1.0) / np.float32(ratio)))
            cconst.append(ct)
            rt = sm.tile([128, KB], F32, tag=f"rc{i}", name=f"rc{i}")
            nc.vector.memset(rt, float(np.float32(ratio)))
            rconst.append(rt)

        # Per-group quant state; bmax alternates tags so group g's sf can
        # still read it after group g+1's prep has started.
        cur = {}

        def emit_wt_load(g):
            # wt shares its buffer with the ratio loop's d tile; emitting
            # the load right after the last d keeps WAR order correct and
            # gives the DMA the whole final pass to complete.
            wt = wtp.tile([128, K], F32, tag="wt", name="wt")
            nc.scalar.dma_start(out=wt, in_=w[g * 128 : (g + 1) * 128, :])
            cur[("wt", g)] = wt

        def emit_prep_rest(g):
            # bmax/inv/b2s/a2s for group g - emitted under group g-1's
            # final pass so the scalar Abs->Au->Ac chain is off the
            # group-boundary critical path.
            wt = cur.pop(("wt", g))
            wt3 = wt.rearrange("p (b e) -> p b e", e=32)
            bmax = sm.tile([128, KB], F32, tag=f"bmax{g % 2}", name="bmax")
            nc.vector.tensor_reduce(
                out=bmax, in_=wt3, axis=mybir.AxisListType.X, op=A.max,
                apply_absolute_value=True,
            )
            nc.vector.tensor_scalar(out=bmax, in0=bmax, scalar1=1e-12, scalar2=None, op0=A.max)
            inv = sm.tile([128, KB], F32, tag="inv", name="inv")
            nc.vector.reciprocal(out=inv, in_=bmax)
            b2s = big.tile([128, K], F32, tag="b2s", name="b2s")
            b2s3 = b2s.rearrange("p (b e) -> p b e", e=32)
            inv_b = inv.unsqueeze(2).broadcast_to([128, KB, 32])
            nc.vector.scalar_tensor_tensor(
                out=b2s3, in0=wt3, scalar=12.0, in1=inv_b, op0=A.mult, op1=A.mult,
            )
            a2s = big.tile([128, K], F32, tag="a2s", name="a2s")
            nc.scalar.activation(out=a2s, in_=b2s, func=AF.Abs)
            cur["bmax"], cur["b2s"], cur["a2s"] = bmax, b2s, a2s
            # pre-warm ratio 0's clip on ScalarE
            cur["Ac0"] = emit_clip(0)

        def emit_clip(i):
            # Ac = min(a2s*c_i, 12) via 12 - Relu(12 - c_i*a2s) on ScalarE
            c = float(np.float32(1.0) / np.float32(RATIOS[i]))
            Au = big.tile([128, K], F32, tag="sE", name="Au")
            nc.scalar.activation(
                out=Au, in_=cur["a2s"], func=AF.Relu, scale=-c, bias=twelve,
            )
            Ac = big.tile([128, K], F32, tag="sA", name="Ac")
            nc.scalar.activation(
                out=Ac, in_=Au, func=AF.Identity, scale=-1.0, bias=twelve,
            )
            return Ac

        def emit_mask(Ac):
            # tta = exp2floor-bits(Ac);  msv = max(tta*MAGIC/2, MAGIC)
            # (identity: max(2^e, 2)*M/2 == max(2^e*M/2, M))
            tta = big.tile([128, K], F32, tag="sB", name="tta")
            nc.vector.tensor_scalar(
                out=tta.bitcast(I32), in0=Ac.bitcast(I32),
                scalar1=EXP_MASK, scalar2=None, op0=A.bitwise_and,
            )
            msv = big.tile([128, K], F32, tag="sD", name="msv")
            nc.vector.tensor_scalar(
                out=msv, in0=tta, scalar1=MAGIC / 2.0, scalar2=MAGIC,
                op0=A.mult, op1=A.max,
            )
            return msv

        def emit_search(g):
            a2s = cur["a2s"]
            best_e = sm.tile([128, KB], F32, tag="best_e", name="best_e")
            nc.vector.memset(best_e, INF)
            best_c = sm.tile([128, KB], F32, tag="best_c", name="best_c")
            nc.vector.memset(best_c, 0.0)
            best_r = sm.tile([128, KB], F32, tag="best_r", name="best_r")
            nc.vector.memset(best_r, 1.0)
            cur["best_c"], cur["best_r"] = best_c, best_r

            # Software-pipelined MSE search: ScalarE computes ratio i+1's
            # clipped operand while DVE rounds ratio i; DVE hides the
            # Square latency under ratio i+1's tta/msv. Zero-stall.
            Ac = cur.pop("Ac0")
            msv = emit_mask(Ac)
            for i, ratio in enumerate(RATIOS):
                c = float(np.float32(1.0) / np.float32(ratio))
                # r = Ac + msv ; q = r - msv  (RNE onto the e2m1 grid)
                r_ = big.tile([128, K], F32, tag="sC", name="r")
                nc.vector.tensor_tensor(out=r_, in0=Ac, in1=msv, op=A.add)
                q_ = big.tile([128, K], F32, tag="sB", name="q")
                nc.vector.scalar_tensor_tensor(
                    out=q_, in0=msv, scalar=-1.0, in1=r_, op0=A.mult, op1=A.add,
                )
                if i + 1 < len(RATIOS):
                    Ac = emit_clip(i + 1)
                # d = a2s*c - q (unclipped error, matches reference MSE);
                # lives in the idle wt buffer
                d_ = wtp.tile([128, K], F32, tag="wt", name="d")
                nc.vector.scalar_tensor_tensor(
                    out=d_, in0=a2s, scalar=c, in1=q_, op0=A.mult, op1=A.subtract,
                )
                if i + 1 < len(RATIOS):
                    msv = emit_mask(Ac)
                # dsq = (ratio*d)^2 : folds the ratio^2 MSE weighting in
                dsq = big.tile([128, K], F32, tag="sC", name="dsq")
                nc.scalar.activation(
                    out=dsq, in_=d_, func=AF.Square, scale=float(np.float32(ratio)),
                )
                e_ = sm.tile([128, KB], F32, tag="e", name="e")
                nc.vector.tensor_reduce(
                    out=e_, in_=dsq.rearrange("p (b e) -> p b e", e=32),
                    axis=mybir.AxisListType.X, op=A.add,
                )
                mask = sm.tile([128, KB], I32, tag="mask", name="mask")
                nc.vector.tensor_tensor(out=mask, in0=e_, in1=best_e, op=A.is_lt)
                nc.vector.tensor_tensor(out=best_e, in0=e_, in1=best_e, op=A.min)
                nc.vector.copy_predicated(out=best_c, mask=mask, data=cconst[i])
                nc.vector.copy_predicated(out=best_r, mask=mask, data=rconst[i])

        def emit_final(g, prefetch_next):
            bmax, b2s = cur["bmax"], cur["b2s"]
            best_c, best_r = cur["best_c"], cur["best_r"]
            # scale factor sf = bmax * best_r / 6, rounded to fp8 e4m3 (RNE,
            # subnormal-aware) emulated in fp32, then halved (q = q2/2).
            sf = sm.tile([128, KB], F32, tag="sf", name="sf")
            nc.vector.scalar_tensor_tensor(
                out=sf, in0=bmax, scalar=1.0 / 6.0, in1=best_r, op0=A.mult, op1=A.mult,
            )
            eb8 = sm.tile([128, KB], F32, tag="eb8", name="eb8")
            nc.vector.tensor_scalar(
                out=eb8.bitcast(I32), in0=sf.bitcast(I32),
                scalar1=EXP_MASK, scalar2=None, op0=A.bitwise_and,
            )
            ms8 = sm.tile([128, KB], F32, tag="ms8", name="ms8")
            nc.vector.tensor_scalar(
                out=ms8, in0=eb8, scalar1=MAGIC8_HI, scalar2=MAGIC8_LO, op0=A.mult, op1=A.max,
            )
            nc.vector.tensor_tensor(out=sf, in0=sf, in1=ms8, op=A.add)
            nc.vector.tensor_tensor(out=sf, in0=sf, in1=ms8, op=A.subtract)
            nc.vector.tensor_scalar(out=sf, in0=sf, scalar1=0.5, scalar2=None, op0=A.mult)

            # final quantization with the chosen scale (signed)
            B2f = big.tile([128, K], F32, tag="sA", name="B2f")
            B2f3 = B2f.rearrange("p (b e) -> p b e", e=32)
            bc_b = best_c.unsqueeze(2).broadcast_to([128, KB, 32])
            nc.vector.tensor_tensor(out=B2f3, in0=b2s.rearrange("p (b e) -> p b e", e=32), in1=bc_b, op=A.mult)
            if prefetch_next:
                # next group's bmax/b2s/a2s + scalar clip chain, hidden
                # under this group's final pass
                emit_prep_rest(g + 1)
            ttaf = big.tile([128, K], F32, tag="sB", name="ttaf")
            nc.vector.tensor_scalar(
                out=ttaf.bitcast(I32), in0=B2f.bitcast(I32),
                scalar1=EXP_MASK, scalar2=None, op0=A.bitwise_and,
            )
            msvf = big.tile([128, K], F32, tag="sD", name="msvf")
            nc.vector.tensor_scalar(
                out=msvf, in0=ttaf, scalar1=MAGIC / 2.0, scalar2=MAGIC,
                op0=A.mult, op1=A.max,
            )
            rf = big.tile([128, K], F32, tag="sC", name="rf")
            nc.vector.tensor_tensor(out=rf, in0=B2f, in1=msvf, op=A.add)
            qf = big.tile([128, K], F32, tag="sE", name="qf")
            nc.vector.scalar_tensor_tensor(
                out=qf, in0=msvf, scalar=-1.0, in1=rf, op0=A.mult, op1=A.add,
            )
            qc = big.tile([128, K], F32, tag="sB", name="qc")
            nc.vector.tensor_scalar(
                out=qc, in0=qf, scalar1=12.0, scalar2=-12.0, op0=A.min, op1=A.max,
            )
            wdq = big.tile([128, K], BF16, tag="wdq", name="wdq")
            sf_b = sf.unsqueeze(2).broadcast_to([128, KB, 32])
            nc.vector.tensor_tensor(
                out=wdq.rearrange("p (b e) -> p b e", e=32),
                in0=qc.rearrange("p (b e) -> p b e", e=32),
                in1=sf_b, op=A.mult,
            )

            # transpose into wdqT[:, kc, g*128:(g+1)*128]
            for kc in range(KC):
                pt = psum.tile([128, 128], BF16, tag="ptr", bufs=2, name="pt")
                nc.tensor.transpose(pt, wdq[:, kc * 128 : (kc + 1) * 128], ident)
                nc.scalar.copy(out=wdqT[:, kc, g * 128 : (g + 1) * 128], in_=pt)

        def emit_quant_group(g):
            emit_search(g)
            if g + 1 < NG:
                emit_wt_load(g + 1)
            emit_final(g, prefetch_next=(g + 1 < NG))

        # Matmul runs over GROUP-PAIRS: each xt tile feeds 4 MMs (2 groups
        # x 2 m-groups), halving DMA bytes per MM vs per-group passes.
        # Pair {0,1} hides under quant of groups 2,3; pair {2,3} is the tail.
        pair_psums = {}

        def emit_pair_mms(pair, rings, inline_drain=False):
            g0 = 2 * pair
            pair_psums[pair] = []
            for mc in range(M // 1024):
                psums = [
                    psum.tile([128, 512], F32, tag=f"pp{j}", name=f"pp{j}")
                    for j in range(4)
                ]
                pair_psums[pair].append(psums)
                for kc in range(KC):
                    xt = xtp.tile([128, 1024], BF16, tag="xt", name="xt")
                    rings[(mc * KC + kc) % len(rings)].dma_start(
                        out=xt,
                        in_=xT[kc * 128 : (kc + 1) * 128,
                               mc * 1024 : (mc + 1) * 1024],
                    )
                    for gi in range(2):
                        for j in range(2):
                            nc.tensor.matmul(
                                psums[gi * 2 + j],
                                lhsT=wdqT[:, kc, (g0 + gi) * 128 : (g0 + gi + 1) * 128],
                                rhs=xt[:, j * 512 : (j + 1) * 512],
                                start=(kc == 0),
                                stop=(kc == KC - 1),
                            )
                if inline_drain:
                    emit_chunk_drain(g0, mc, psums)

        def emit_chunk_drain(g0, mc, psums):
            for gi in range(2):
                for j in range(2):
                    ysb = ytp.tile([128, 512], F32, tag="ysb", name="ysb")
                    nc.scalar.add(out=ysb, in_=psums[gi * 2 + j], add=bias_sb[g0 + gi])
                    g, mg = g0 + gi, mc * 2 + j
                    nc.sync.dma_start(
                        out=yT[g * 128 : (g + 1) * 128, mg * 512 : (mg + 1) * 512],
                        in_=ysb,
                    )

        def emit_pair_tail(pair):
            # bias-add drain on ScalarE - only ever emitted after all quant
            # scalar work, so it cannot stall the quant chain.
            g0 = 2 * pair
            for mc, psums in enumerate(pair_psums[pair]):
                for gi in range(2):
                    for j in range(2):
                        ysb = ytp.tile([128, 512], F32, tag="ysb", name="ysb")
                        nc.scalar.add(out=ysb, in_=psums[gi * 2 + j], add=bias_sb[g0 + gi])
                        g, mg = g0 + gi, mc * 2 + j
                        nc.sync.dma_start(
                            out=yT[g * 128 : (g + 1) * 128, mg * 512 : (mg + 1) * 512],
                            in_=ysb,
                        )

        emit_wt_load(0)
        emit_prep_rest(0)
        emit_quant_group(0)
        emit_quant_group(1)
        emit_pair_mms(0, [nc.sync, nc.gpsimd])
        emit_quant_group(2)
        emit_quant_group(3)
        emit_pair_tail(0)
        emit_pair_mms(1, [nc.sync, nc.gpsimd], inline_drain=True)

    nc.compile()
    return nc


_NC_CACHE = None


def _in_maps(x, weight, bias):
    import ml_dtypes

    x = np.ascontiguousarray(x, dtype=np.float32)
    weight = np.ascontiguousarray(weight, dtype=np.float32)
    bias = np.ascontiguousarray(bias, dtype=np.float32)
    xT = np.ascontiguousarray(x.T).astype(ml_dtypes.bfloat16)
    in_maps = []
    for c in range(NCORES):
        in_maps.append(
            {
                "xT": xT,
                "w": weight[c * NSH : (c + 1) * NSH],
                "bias": bias[c * NSH : (c + 1) * NSH].reshape(NSH, 1),
            }
        )
    return in_maps


def kernel(x: np.ndarray, weight: np.ndarray, bias: np.ndarray) -> np.ndarray:
    global _NC_CACHE
    if _NC_CACHE is None:
        _NC_CACHE = build_nc()
    nc = _NC_CACHE
    res = run_bass_kernel_spmd(nc, _in_maps(x, weight, bias), list(range(NCORES)))
    yT = np.concatenate([res.results[c]["yT"] for c in range(NCORES)], axis=0)
    return np.ascontiguousarray(yT.T)


def profile_once(x, weight, bias):
    global _NC_CACHE
    if _NC_CACHE is None:
        _NC_CACHE = build_nc()
    nc = _NC_CACHE
    res = run_bass_kernel_spmd(
        nc, _in_maps(x, weight, bias), list(range(NCORES)),
        trace=True, tmpdir="/tmp/nvfp4_trace",
    )
    print("exec_time_ns:", res.exec_time_ns, "mean:", res.mean_exec_time_ns,
          "max_core:", res.max_exec_time_core_id)
    return res.exec_time_ns


# revision 23
# speedup vs baseline: 1.5063x; 1.0085x over previous
"""NVFP4 fake-quantized linear layer on 8 Trainium2 NeuronCores.

Computes: y = x @ dequant(nvfp4_quantize(weight)).T + bias
  x [8192, 4096] f32, weight [4096, 4096] f32, bias [4096] f32.

Strategy (tensor-parallel, row-wise weight sharding, 512 rows/core):
  - x is transposed and cast to bf16 on the host (layout/precision prep);
    every core receives the full xT [K, M] so the matmul phase streams
    natural [128k, m] tiles with plain DMA - no transpose engine, no
    AllGather on the critical path.
  - Quantization (per-(row, 32-block) MSE scale search, bit-faithful fp32
    incl. fp8-e4m3 scale rounding emulated in fp32) is software-pipelined
    across engines with zero-stall buffer rotation:
      ScalarE (ratio i+1): Au = Relu(12 - c*a2s); Ac = 12 - Au  (clip)
      DVE     (ratio i):   tta = Ac & EXP_MASK (exp2 floor bits)
                           msv = max(tta*MAGIC/2, MAGIC)  (fused clamp:
                                 max(2^e,2)*M/2 == max(2^e*M/2, M))
                           r = Ac + msv ; q = r - msv  (magic RNE round)
                           d = a2s*c - q  (unclipped error -> wt buffer)
      ScalarE: dsq = Square(ratio*d)  (ratio^2 MSE weighting folded in)
      DVE:     e = per-32-block reduce(dsq); argmin bookkeeping
    TensorE transposes the dequantized weights into wdqT bf16.
  - Matmul runs over GROUP-PAIRS so each xt tile feeds 4 MMs (2 groups x
    2 m-groups), halving DMA bytes per matmul: pair {0,1} executes hidden
    under quantization of groups 2,3; pair {2,3} is the tail with inline
    per-chunk psum drains. Bias is added during the PSUM->SBUF drain; each
    core writes yT [512, 8192] and the host concatenates + transposes.
"""

import sys

sys.path.insert(0, "/opt/trn_rl_repo")

from contextlib import ExitStack

import numpy as np

import concourse.bass as bass
import concourse.bacc as bacc
import concourse.tile as tile
from concourse import mybir
from concourse.bass_utils import run_bass_kernel_spmd

A = mybir.AluOpType
AF = mybir.ActivationFunctionType
F32 = mybir.dt.float32
BF16 = mybir.dt.bfloat16
I32 = mybir.dt.int32

NCORES = 8
M, K, N = 8192, 4096, 4096
NSH = N // NCORES          # 512 weight rows per core
NG = NSH // 128            # 4 row groups per core
KC = K // 128              # 32 contraction chunks
KB = K // 32               # 128 blocks per weight row
MG = M // 512              # 16 output m-groups
MCH = 4                    # m-groups per psum chunk (4 banks live)

RATIOS = [float(r) for r in np.linspace(0.7, 1.0, 10)]
MAGIC = 12582912.0         # 1.5 * 2**23 : RNE-to-integer magic constant
INF = float("inf")
EXP_MASK = 0x7F800000      # fp32 exponent field mask
ABS_MASK = 0x7FFFFFFF      # clears the sign bit
TWO_BITS = 0x40000000      # bits of 2.0f; int max == float max for positives
# fp8 e4m3 rounding grid: step = max(2^-9, exp2floor(x) * 2^-3)
MAGIC8_HI = MAGIC / 8.0
MAGIC8_LO = MAGIC / 512.0


def build_nc() -> bass.Bass:
    nc = bacc.Bacc("TRN2", num_devices=NCORES)

    xT = nc.declare_dram_parameter("xT", [K, M], BF16, isOutput=False)
    w = nc.declare_dram_parameter("w", [NSH, K], F32, isOutput=False)
    bias = nc.declare_dram_parameter("bias", [NSH, 1], F32, isOutput=False)
    yT = nc.declare_dram_parameter("yT", [NSH, M], F32, isOutput=True)

    with tile.TileContext(nc) as tc, ExitStack() as ctx:
        big = ctx.enter_context(tc.tile_pool(name="big", bufs=1))
        sm = ctx.enter_context(tc.tile_pool(name="small", bufs=1))
        wtp = ctx.enter_context(tc.tile_pool(name="wtp", bufs=1))
        psum = ctx.enter_context(tc.tile_pool(name="psum", bufs=1, space="PSUM"))
        xtp = ctx.enter_context(tc.tile_pool(name="xtp", bufs=8))
        ytp = ctx.enter_context(tc.tile_pool(name="ytp", bufs=2))

        # persistent w_dq^T, bf16 [128 k-partitions, 32 k-chunks, 512 n]
        wdqT = big.tile([128, KC, NSH], BF16, tag="wdqT", name="wdqT")

        ident = sm.tile([128, 128], BF16, tag="ident", name="ident")
        from concourse.masks import make_identity

        make_identity(nc, ident)

        bias_sb = []
        for g in range(NG):
            bsl = sm.tile([128, 1], F32, tag=f"bias{g}", name=f"bias{g}")
            nc.scalar.dma_start(out=bsl, in_=bias[g * 128 : (g + 1) * 128, :])
            bias_sb.append(bsl)

        twelve = sm.tile([128, 1], F32, tag="twelve", name="twelve")
        nc.vector.memset(twelve, 12.0)

        # hoisted per-ratio constant tiles for the argmin bookkeeping
        cconst, rconst = [], []
        for i, ratio in enumerate(RATIOS):
            ct = sm.tile([128, KB], F32, tag=f"cc{i}", name=f"cc{i}")
            nc.vector.memset(ct, float(np.float32(1.0) / np.float32(ratio)))
            cconst.append(ct)
            rt = sm.tile([128, KB], F32, tag=f"rc{i}", name=f"rc{i}")
            nc.vector.memset(rt, float(np.float32(ratio)))
            rconst.append(rt)

        # Per-group quant state; bmax alternates tags so group g's sf can
        # still read it after group g+1's prep has started.
        cur = {}

        def emit_wt_load(g):
            # wt shares its buffer with the ratio loop's d tile; emitting
            # the load right after the last d keeps WAR order correct and
            # gives the DMA the whole final pass to complete.
            wt = wtp.tile([128, K], F32, tag="wt", name="wt")
            nc.scalar.dma_start(out=wt, in_=w[g * 128 : (g + 1) * 128, :])
            cur[("wt", g)] = wt

        def emit_prep_rest(g):
            # bmax/inv/b2s/a2s for group g - emitted under group g-1's
            # final pass so the scalar Abs->Au->Ac chain is off the
            # group-boundary critical path.
            wt = cur.pop(("wt", g))
            wt3 = wt.rearrange("p (b e) -> p b e", e=32)
            bmax = sm.tile([128, KB], F32, tag=f"bmax{g % 2}", name="bmax")
            nc.vector.tensor_reduce(
                out=bmax, in_=wt3, axis=mybir.AxisListType.X, op=A.max,
                apply_absolute_value=True,
            )
            nc.vector.tensor_scalar(out=bmax, in0=bmax, scalar1=1e-12, scalar2=None, op0=A.max)
            inv = sm.tile([128, KB], F32, tag="inv", name="inv")
            nc.vector.reciprocal(out=inv, in_=bmax)
            b2s = big.tile([128, K], F32, tag="b2s", name="b2s")
            b2s3 = b2s.rearrange("p (b e) -> p b e", e=32)
            inv_b = inv.unsqueeze(2).broadcast_to([128, KB, 32])
            nc.vector.scalar_tensor_tensor(
                out=b2s3, in0=wt3, scalar=12.0, in1=inv_b, op0=A.mult, op1=A.mult,
            )
            a2s = big.tile([128, K], F32, tag="a2s", name="a2s")
            nc.scalar.activation(out=a2s, in_=b2s, func=AF.Abs)
            cur["bmax"], cur["b2s"], cur["a2s"] = bmax, b2s, a2s
            # pre-warm ratio 0's clip on ScalarE
            cur["Ac0"] = emit_clip(0)

        def emit_clip(i):
            # Ac = min(a2s*c_i, 12) via 12 - Relu(12 - c_i*a2s) on ScalarE
            c = float(np.float32(1.0) / np.float32(RATIOS[i]))
            Au = big.tile([128, K], F32, tag="sE", name="Au")
            nc.scalar.activation(
                out=Au, in_=cur["a2s"], func=AF.Relu, scale=-c, bias=twelve,
            )
            Ac = big.tile([128, K], F32, tag="sA", name="Ac")
            nc.scalar.activation(
                out=Ac, in_=Au, func=AF.Identity, scale=-1.0, bias=twelve,
            )
            return Ac

        def emit_mask(Ac):
            # tta = exp2floor-bits(Ac);  msv = max(tta*MAGIC/2, MAGIC)
            # (identity: max(2^e, 2)*M/2 == max(2^e*M/2, M))
            tta = big.tile([128, K], F32, tag="sB", name="tta")
            nc.vector.tensor_scalar(
                out=tta.bitcast(I32), in0=Ac.bitcast(I32),
                scalar1=EXP_MASK, scalar2=None, op0=A.bitwise_and,
            )
            msv = big.tile([128, K], F32, tag="sD", name="msv")
            nc.vector.tensor_scalar(
                out=msv, in0=tta, scalar1=MAGIC / 2.0, scalar2=MAGIC,
                op0=A.mult, op1=A.max,
            )
            return msv

        def emit_search(g):
            a2s = cur["a2s"]
            best_e = sm.tile([128, KB], F32, tag="best_e", name="best_e")
            nc.vector.memset(best_e, INF)
            best_c = sm.tile([128, KB], F32, tag="best_c", name="best_c")
            nc.vector.memset(best_c, 0.0)
            best_r = sm.tile([128, KB], F32, tag="best_r", name="best_r")
            nc.vector.memset(best_r, 1.0)
            cur["best_c"], cur["best_r"] = best_c, best_r

            # Software-pipelined MSE search: ScalarE computes ratio i+1's
            # clipped operand while DVE rounds ratio i; DVE hides the
            # Square latency under ratio i+1's tta/msv. Zero-stall.
            Ac = cur.pop("Ac0")
            msv = emit_mask(Ac)
            for i, ratio in enumerate(RATIOS):
                c = float(np.float32(1.0) / np.float32(ratio))
                # r = Ac + msv ; q = r - msv  (RNE onto the e2m1 grid)
                r_ = big.tile([128, K], F32, tag="sC", name="r")
                nc.vector.tensor_tensor(out=r_, in0=Ac, in1=msv, op=A.add)
                q_ = big.tile([128, K], F32, tag="sB", name="q")
                nc.vector.scalar_tensor_tensor(
                    out=q_, in0=msv, scalar=-1.0, in1=r_, op0=A.mult, op1=A.add,
                )
                if i + 1 < len(RATIOS):
                    Ac = emit_clip(i + 1)
                # d = a2s*c - q (unclipped error, matches reference MSE);
                # lives in the idle wt buffer
                d_ = wtp.tile([128, K], F32, tag="wt", name="d")
                nc.vector.scalar_tensor_tensor(
                    out=d_, in0=a2s, scalar=c, in1=q_, op0=A.mult, op1=A.subtract,
                )
                if i + 1 < len(RATIOS):
                    msv = emit_mask(Ac)
                # dsq = (ratio*d)^2 : folds the ratio^2 MSE weighting in
                dsq = big.tile([128, K], F32, tag="sC", name="dsq")
                nc.scalar.activation(
                    out=dsq, in_=d_, func=AF.Square, scale=float(np.float32(ratio)),
                )
                e_ = sm.tile([128, KB], F32, tag="e", name="e")
                nc.vector.tensor_reduce(
                    out=e_, in_=dsq.rearrange("p (b e) -> p b e", e=32),
                    axis=mybir.AxisListType.X, op=A.add,
                )
                mask = sm.tile([128, KB], I32, tag="mask", name="mask")
                nc.vector.tensor_tensor(out=mask, in0=e_, in1=best_e, op=A.is_lt)
                nc.vector.tensor_tensor(out=best_e, in0=e_, in1=best_e, op=A.min)
                nc.vector.copy_predicated(out=best_c, mask=mask, data=cconst[i])
                nc.vector.copy_predicated(out=best_r, mask=mask, data=rconst[i])

        def emit_final(g, prefetch_next):
            bmax, b2s = cur["bmax"], cur["b2s"]
            best_c, best_r = cur["best_c"], cur["best_r"]
            # scale factor sf = bmax * best_r / 6, rounded to fp8 e4m3 (RNE,
            # subnormal-aware) emulated in fp32, then halved (q = q2/2).
            sf = sm.tile([128, KB], F32, tag="sf", name="sf")
            nc.vector.scalar_tensor_tensor(
                out=sf, in0=bmax, scalar=1.0 / 6.0, in1=best_r, op0=A.mult, op1=A.mult,
            )
            eb8 = sm.tile([128, KB], F32, tag="eb8", name="eb8")
            nc.vector.tensor_scalar(
                out=eb8.bitcast(I32), in0=sf.bitcast(I32),
                scalar1=EXP_MASK, scalar2=None, op0=A.bitwise_and,
            )
            ms8 = sm.tile([128, KB], F32, tag="ms8", name="ms8")
            nc.vector.tensor_scalar(
                out=ms8, in0=eb8, scalar1=MAGIC8_HI, scalar2=MAGIC8_LO, op0=A.mult, op1=A.max,
            )
            nc.vector.tensor_tensor(out=sf, in0=sf, in1=ms8, op=A.add)
            nc.vector.tensor_tensor(out=sf, in0=sf, in1=ms8, op=A.subtract)
            nc.vector.tensor_scalar(out=sf, in0=sf, scalar1=0.5, scalar2=None, op0=A.mult)

            # final quantization with the chosen scale (signed)
            B2f = big.tile([128, K], F32, tag="sA", name="B2f")
            B2f3 = B2f.rearrange("p (b e) -> p b e", e=32)
            bc_b = best_c.unsqueeze(2).broadcast_to([128, KB, 32])
            nc.vector.tensor_tensor(out=B2f3, in0=b2s.rearrange("p (b e) -> p b e", e=32), in1=bc_b, op=A.mult)
            if prefetch_next:
                # next group's bmax/b2s/a2s + scalar clip chain, hidden
                # under this group's final pass
                emit_prep_rest(g + 1)
            ttaf = big.tile([128, K], F32, tag="sB", name="ttaf")
            nc.vector.tensor_scalar(
                out=ttaf.bitcast(I32), in0=B2f.bitcast(I32),
                scalar1=EXP_MASK, scalar2=None, op0=A.bitwise_and,
            )
            msvf = big.tile([128, K], F32, tag="sD", name="msvf")
            nc.vector.tensor_scalar(
                out=msvf, in0=ttaf, scalar1=MAGIC / 2.0, scalar2=MAGIC,
                op0=A.mult, op1=A.max,
            )
            rf = big.tile([128, K], F32, tag="sC", name="rf")
            nc.vector.tensor_tensor(out=rf, in0=B2f, in1=msvf, op=A.add)
            qf = big.tile([128, K], F32, tag="sE", name="qf")
            nc.vector.scalar_tensor_tensor(
                out=qf, in0=msvf, scalar=-1.0, in1=rf, op0=A.mult, op1=A.add,
            )
            qc = big.tile([128, K], F32, tag="sB", name="qc")
            nc.vector.tensor_scalar(
                out=qc, in0=qf, scalar1=12.0, scalar2=-12.0, op0=A.min, op1=A.max,
            )
            wdq = big.tile([128, K], BF16, tag="wdq", name="wdq")
            sf_b = sf.unsqueeze(2).broadcast_to([128, KB, 32])
            nc.vector.tensor_tensor(
                out=wdq.rearrange("p (b e) -> p b e", e=32),
                in0=qc.rearrange("p (b e) -> p b e", e=32),
                in1=sf_b, op=A.mult,
            )

            # transpose into wdqT[:, kc, g*128:(g+1)*128]
            for kc in range(KC):
                pt = psum.tile([128, 128], BF16, tag="ptr", bufs=2, name="pt")
                nc.tensor.transpose(pt, wdq[:, kc * 128 : (kc + 1) * 128], ident)
                nc.scalar.copy(out=wdqT[:, kc, g * 128 : (g + 1) * 128], in_=pt)

        def emit_quant_group(g):
            emit_search(g)
            if g + 1 < NG:
                emit_wt_load(g + 1)
            emit_final(g, prefetch_next=(g + 1 < NG))

        # Matmul runs over GROUP-PAIRS: each xt tile feeds 4 MMs (2 groups
        # x 2 m-groups), halving DMA bytes per MM vs per-group passes.
        # Pair {0,1} hides under quant of groups 2,3; pair {2,3} is the tail.
        pair_psums = {}

        def emit_pair_mms(pair, rings, inline_drain=False):
            g0 = 2 * pair
            pair_psums[pair] = []
            for mc in range(M // 1024):
                psums = [
                    psum.tile([128, 512], F32, tag=f"pp{j}", name=f"pp{j}")
                    for j in range(4)
                ]
                pair_psums[pair].append(psums)
                for kc in range(KC):
                    xt = xtp.tile([128, 1024], BF16, tag="xt", name="xt")
                    rings[(mc * KC + kc) % len(rings)].dma_start(
                        out=xt,
                        in_=xT[kc * 128 : (kc + 1) * 128,
                               mc * 1024 : (mc + 1) * 1024],
                    )
                    for gi in range(2):
                        for j in range(2):
                            nc.tensor.matmul(
                                psums[gi * 2 + j],
                                lhsT=wdqT[:, kc, (g0 + gi) * 128 : (g0 + gi + 1) * 128],
                                rhs=xt[:, j * 512 : (j + 1) * 512],
                                start=(kc == 0),
                                stop=(kc == KC - 1),
                            )
                if inline_drain:
                    emit_chunk_drain(g0, mc, psums)

        def emit_chunk_drain(g0, mc, psums):
            for gi in range(2):
                for j in range(2):
                    ysb = ytp.tile([128, 512], F32, tag="ysb", name="ysb")
                    nc.scalar.add(out=ysb, in_=psums[gi * 2 + j], add=bias_sb[g0 + gi])
                    g, mg = g0 + gi, mc * 2 + j
                    nc.sync.dma_start(
                        out=yT[g * 128 : (g + 1) * 128, mg * 512 : (mg + 1) * 512],
                        in_=ysb,
                    )

        def emit_pair_tail(pair):
            # bias-add drain on the DVE: emitted after all quant, so it
            # executes during the tail matmul window where DVE is idle -
            # and the scheduler cannot hoist it into the Scalar engine's
            # quant-critical Au/Ac chain.
            g0 = 2 * pair
            for mc, psums in enumerate(pair_psums[pair]):
                for gi in range(2):
                    for j in range(2):
                        ysb = ytp.tile([128, 512], F32, tag="ysb", name="ysb")
                        nc.vector.tensor_scalar(
                            out=ysb, in0=psums[gi * 2 + j],
                            scalar1=bias_sb[g0 + gi], scalar2=None, op0=A.add,
                        )
                        g, mg = g0 + gi, mc * 2 + j
                        nc.sync.dma_start(
                            out=yT[g * 128 : (g + 1) * 128, mg * 512 : (mg + 1) * 512],
                            in_=ysb,
                        )

        emit_wt_load(0)
        emit_prep_rest(0)
        emit_quant_group(0)
        emit_quant_group(1)
        emit_pair_mms(0, [nc.sync, nc.gpsimd])
        emit_quant_group(2)
        emit_quant_group(3)
        emit_pair_tail(0)
        emit_pair_mms(1, [nc.sync, nc.gpsimd], inline_drain=True)

    nc.compile()
    return nc


_NC_CACHE = None


def _in_maps(x, weight, bias):
    import ml_dtypes

    x = np.ascontiguousarray(x, dtype=np.float32)
    weight = np.ascontiguousarray(weight, dtype=np.float32)
    bias = np.ascontiguousarray(bias, dtype=np.float32)
    xT = np.ascontiguousarray(x.T).astype(ml_dtypes.bfloat16)
    in_maps = []
    for c in range(NCORES):
        in_maps.append(
            {
                "xT": xT,
                "w": weight[c * NSH : (c + 1) * NSH],
                "bias": bias[c * NSH : (c + 1) * NSH].reshape(NSH, 1),
            }
        )
    return in_maps


def kernel(x: np.ndarray, weight: np.ndarray, bias: np.ndarray) -> np.ndarray:
    global _NC_CACHE
    if _NC_CACHE is None:
        _NC_CACHE = build_nc()
    nc = _NC_CACHE
    res = run_bass_kernel_spmd(nc, _in_maps(x, weight, bias), list(range(NCORES)))
    yT = np.concatenate([res.results[c]["yT"] for c in range(NCORES)], axis=0)
    return np.ascontiguousarray(yT.T)


def profile_once(x, weight, bias):
    global _NC_CACHE
    if _NC_CACHE is None:
        _NC_CACHE = build_nc()
    nc = _NC_CACHE
    res = run_bass_kernel_spmd(
        nc, _in_maps(x, weight, bias), list(range(NCORES)),
        trace=True, tmpdir="/tmp/nvfp4_trace",
    )
    print("exec_time_ns:", res.exec_time_ns, "mean:", res.mean_exec_time_ns,
          "max_core:", res.max_exec_time_core_id)
    return res.exec_time_ns


# revision 24
# speedup vs baseline: 1.5150x; 1.0058x over previous
"""NVFP4 fake-quantized linear layer on 8 Trainium2 NeuronCores.

Computes: y = x @ dequant(nvfp4_quantize(weight)).T + bias
  x [8192, 4096] f32, weight [4096, 4096] f32, bias [4096] f32.

Strategy (tensor-parallel, row-wise weight sharding, 512 rows/core):
  - x is transposed and cast to bf16 on the host (layout/precision prep);
    every core receives the full xT [K, M] so the matmul phase streams
    natural [128k, m] tiles with plain DMA - no transpose engine, no
    AllGather on the critical path.
  - Quantization (per-(row, 32-block) MSE scale search, bit-faithful fp32
    incl. fp8-e4m3 scale rounding emulated in fp32) is software-pipelined
    across engines with zero-stall buffer rotation:
      ScalarE (ratio i+1): Au = Relu(12 - c*a2s); Ac = 12 - Au  (clip)
      DVE     (ratio i):   tta = Ac & EXP_MASK (exp2 floor bits)
                           msv = max(tta*MAGIC/2, MAGIC)  (fused clamp:
                                 max(2^e,2)*M/2 == max(2^e*M/2, M))
                           r = Ac + msv ; q = r - msv  (magic RNE round)
                           d = a2s*c - q  (unclipped error -> wt buffer)
      ScalarE: dsq = Square(ratio*d)  (ratio^2 MSE weighting folded in)
      DVE:     e = per-32-block reduce(dsq); argmin bookkeeping
    TensorE transposes the dequantized weights into wdqT bf16.
  - Matmul runs over GROUP-PAIRS so each xt tile feeds 4 MMs (2 groups x
    2 m-groups), halving DMA bytes per matmul: pair {0,1} executes hidden
    under quantization of groups 2,3; pair {2,3} is the tail with inline
    per-chunk psum drains. Bias is added during the PSUM->SBUF drain; each
    core writes yT [512, 8192] and the host concatenates + transposes.
"""

import sys

sys.path.insert(0, "/opt/trn_rl_repo")

from contextlib import ExitStack

import numpy as np

import concourse.bass as bass
import concourse.bacc as bacc
import concourse.tile as tile
from concourse import mybir
from concourse.bass_utils import run_bass_kernel_spmd

A = mybir.AluOpType
AF = mybir.ActivationFunctionType
F32 = mybir.dt.float32
BF16 = mybir.dt.bfloat16
I32 = mybir.dt.int32

NCORES = 8
M, K, N = 8192, 4096, 4096
NSH = N // NCORES          # 512 weight rows per core
NG = NSH // 128            # 4 row groups per core
KC = K // 128              # 32 contraction chunks
KB = K // 32               # 128 blocks per weight row
MG = M // 512              # 16 output m-groups
MCH = 4                    # m-groups per psum chunk (4 banks live)

RATIOS = [float(r) for r in np.linspace(0.7, 1.0, 10)]
MAGIC = 12582912.0         # 1.5 * 2**23 : RNE-to-integer magic constant
INF = float("inf")
EXP_MASK = 0x7F800000      # fp32 exponent field mask
ABS_MASK = 0x7FFFFFFF      # clears the sign bit
TWO_BITS = 0x40000000      # bits of 2.0f; int max == float max for positives
# fp8 e4m3 rounding grid: step = max(2^-9, exp2floor(x) * 2^-3)
MAGIC8_HI = MAGIC / 8.0
MAGIC8_LO = MAGIC / 512.0


def build_nc() -> bass.Bass:
    nc = bacc.Bacc("TRN2", num_devices=NCORES)

    xT = nc.declare_dram_parameter("xT", [K, M], BF16, isOutput=False)
    w = nc.declare_dram_parameter("w", [NSH, K], F32, isOutput=False)
    bias = nc.declare_dram_parameter("bias", [NSH, 1], F32, isOutput=False)
    yT = nc.declare_dram_parameter("yT", [NSH, M], F32, isOutput=True)

    with tile.TileContext(nc) as tc, ExitStack() as ctx:
        big = ctx.enter_context(tc.tile_pool(name="big", bufs=1))
        sm = ctx.enter_context(tc.tile_pool(name="small", bufs=1))
        wtp = ctx.enter_context(tc.tile_pool(name="wtp", bufs=1))
        psum = ctx.enter_context(tc.tile_pool(name="psum", bufs=1, space="PSUM"))
        xtp = ctx.enter_context(tc.tile_pool(name="xtp", bufs=8))
        ytp = ctx.enter_context(tc.tile_pool(name="ytp", bufs=2))

        # persistent w_dq^T, bf16 [128 k-partitions, 32 k-chunks, 512 n]
        wdqT = big.tile([128, KC, NSH], BF16, tag="wdqT", name="wdqT")

        ident = sm.tile([128, 128], BF16, tag="ident", name="ident")
        from concourse.masks import make_identity

        make_identity(nc, ident)

        first_wt = wtp.tile([128, K], F32, tag="wt", name="wt")
        nc.scalar.dma_start(out=first_wt, in_=w[0:128, :])

        bias_sb = []
        for g in range(NG):
            bsl = sm.tile([128, 1], F32, tag=f"bias{g}", name=f"bias{g}")
            nc.scalar.dma_start(out=bsl, in_=bias[g * 128 : (g + 1) * 128, :])
            bias_sb.append(bsl)

        twelve = sm.tile([128, 1], F32, tag="twelve", name="twelve")
        nc.vector.memset(twelve, 12.0)

        # hoisted per-ratio constant tiles for the argmin bookkeeping
        cconst, rconst = [], []
        for i, ratio in enumerate(RATIOS):
            ct = sm.tile([128, KB], F32, tag=f"cc{i}", name=f"cc{i}")
            nc.vector.memset(ct, float(np.float32(1.0) / np.float32(ratio)))
            cconst.append(ct)
            rt = sm.tile([128, KB], F32, tag=f"rc{i}", name=f"rc{i}")
            nc.vector.memset(rt, float(np.float32(ratio)))
            rconst.append(rt)

        # Per-group quant state; bmax alternates tags so group g's sf can
        # still read it after group g+1's prep has started.
        cur = {}

        def emit_wt_load(g):
            # wt shares its buffer with the ratio loop's d tile; emitting
            # the load right after the last d keeps WAR order correct and
            # gives the DMA the whole final pass to complete.
            wt = wtp.tile([128, K], F32, tag="wt", name="wt")
            nc.scalar.dma_start(out=wt, in_=w[g * 128 : (g + 1) * 128, :])
            cur[("wt", g)] = wt

        def emit_prep_rest(g):
            # bmax/inv/b2s/a2s for group g - emitted under group g-1's
            # final pass so the scalar Abs->Au->Ac chain is off the
            # group-boundary critical path.
            wt = cur.pop(("wt", g))
            wt3 = wt.rearrange("p (b e) -> p b e", e=32)
            bmax = sm.tile([128, KB], F32, tag=f"bmax{g % 2}", name="bmax")
            nc.vector.tensor_reduce(
                out=bmax, in_=wt3, axis=mybir.AxisListType.X, op=A.max,
                apply_absolute_value=True,
            )
            nc.vector.tensor_scalar(out=bmax, in0=bmax, scalar1=1e-12, scalar2=None, op0=A.max)
            inv = sm.tile([128, KB], F32, tag="inv", name="inv")
            nc.vector.reciprocal(out=inv, in_=bmax)
            b2s = big.tile([128, K], F32, tag="b2s", name="b2s")
            b2s3 = b2s.rearrange("p (b e) -> p b e", e=32)
            inv_b = inv.unsqueeze(2).broadcast_to([128, KB, 32])
            nc.vector.scalar_tensor_tensor(
                out=b2s3, in0=wt3, scalar=12.0, in1=inv_b, op0=A.mult, op1=A.mult,
            )
            a2s = big.tile([128, K], F32, tag="a2s", name="a2s")
            nc.scalar.activation(out=a2s, in_=b2s, func=AF.Abs)
            cur["bmax"], cur["b2s"], cur["a2s"] = bmax, b2s, a2s
            # pre-warm ratio 0's clip on ScalarE
            cur["Ac0"] = emit_clip(0)

        def emit_clip(i):
            # Ac = min(a2s*c_i, 12) via 12 - Relu(12 - c_i*a2s) on ScalarE
            c = float(np.float32(1.0) / np.float32(RATIOS[i]))
            Au = big.tile([128, K], F32, tag="sE", name="Au")
            nc.scalar.activation(
                out=Au, in_=cur["a2s"], func=AF.Relu, scale=-c, bias=twelve,
            )
            Ac = big.tile([128, K], F32, tag="sA", name="Ac")
            nc.scalar.activation(
                out=Ac, in_=Au, func=AF.Identity, scale=-1.0, bias=twelve,
            )
            return Ac

        def emit_mask(Ac):
            # tta = exp2floor-bits(Ac);  msv = max(tta*MAGIC/2, MAGIC)
            # (identity: max(2^e, 2)*M/2 == max(2^e*M/2, M))
            tta = big.tile([128, K], F32, tag="sB", name="tta")
            nc.vector.tensor_scalar(
                out=tta.bitcast(I32), in0=Ac.bitcast(I32),
                scalar1=EXP_MASK, scalar2=None, op0=A.bitwise_and,
            )
            msv = big.tile([128, K], F32, tag="sD", name="msv")
            nc.vector.tensor_scalar(
                out=msv, in0=tta, scalar1=MAGIC / 2.0, scalar2=MAGIC,
                op0=A.mult, op1=A.max,
            )
            return msv

        def emit_search(g):
            a2s = cur["a2s"]
            best_e = sm.tile([128, KB], F32, tag="best_e", name="best_e")
            nc.vector.memset(best_e, INF)
            best_c = sm.tile([128, KB], F32, tag="best_c", name="best_c")
            nc.vector.memset(best_c, 0.0)
            best_r = sm.tile([128, KB], F32, tag="best_r", name="best_r")
            nc.vector.memset(best_r, 1.0)
            cur["best_c"], cur["best_r"] = best_c, best_r

            # Software-pipelined MSE search: ScalarE computes ratio i+1's
            # clipped operand while DVE rounds ratio i; DVE hides the
            # Square latency under ratio i+1's tta/msv. Zero-stall.
            Ac = cur.pop("Ac0")
            msv = emit_mask(Ac)
            for i, ratio in enumerate(RATIOS):
                c = float(np.float32(1.0) / np.float32(ratio))
                # r = Ac + msv ; q = r - msv  (RNE onto the e2m1 grid)
                r_ = big.tile([128, K], F32, tag="sC", name="r")
                nc.vector.tensor_tensor(out=r_, in0=Ac, in1=msv, op=A.add)
                q_ = big.tile([128, K], F32, tag="sB", name="q")
                nc.vector.scalar_tensor_tensor(
                    out=q_, in0=msv, scalar=-1.0, in1=r_, op0=A.mult, op1=A.add,
                )
                if i + 1 < len(RATIOS):
                    Ac = emit_clip(i + 1)
                # d = a2s*c - q (unclipped error, matches reference MSE);
                # lives in the idle wt buffer
                d_ = wtp.tile([128, K], F32, tag="wt", name="d")
                nc.vector.scalar_tensor_tensor(
                    out=d_, in0=a2s, scalar=c, in1=q_, op0=A.mult, op1=A.subtract,
                )
                if i + 1 < len(RATIOS):
                    msv = emit_mask(Ac)
                # dsq = (ratio*d)^2 : folds the ratio^2 MSE weighting in
                dsq = big.tile([128, K], F32, tag="sC", name="dsq")
                nc.scalar.activation(
                    out=dsq, in_=d_, func=AF.Square, scale=float(np.float32(ratio)),
                )
                e_ = sm.tile([128, KB], F32, tag="e", name="e")
                nc.vector.tensor_reduce(
                    out=e_, in_=dsq.rearrange("p (b e) -> p b e", e=32),
                    axis=mybir.AxisListType.X, op=A.add,
                )
                mask = sm.tile([128, KB], I32, tag="mask", name="mask")
                nc.vector.tensor_tensor(out=mask, in0=e_, in1=best_e, op=A.is_lt)
                nc.vector.tensor_tensor(out=best_e, in0=e_, in1=best_e, op=A.min)
                nc.vector.copy_predicated(out=best_c, mask=mask, data=cconst[i])
                nc.vector.copy_predicated(out=best_r, mask=mask, data=rconst[i])

        def emit_final(g, prefetch_next):
            bmax, b2s = cur["bmax"], cur["b2s"]
            best_c, best_r = cur["best_c"], cur["best_r"]
            # scale factor sf = bmax * best_r / 6, rounded to fp8 e4m3 (RNE,
            # subnormal-aware) emulated in fp32, then halved (q = q2/2).
            sf = sm.tile([128, KB], F32, tag="sf", name="sf")
            nc.vector.scalar_tensor_tensor(
                out=sf, in0=bmax, scalar=1.0 / 6.0, in1=best_r, op0=A.mult, op1=A.mult,
            )
            eb8 = sm.tile([128, KB], F32, tag="eb8", name="eb8")
            nc.vector.tensor_scalar(
                out=eb8.bitcast(I32), in0=sf.bitcast(I32),
                scalar1=EXP_MASK, scalar2=None, op0=A.bitwise_and,
            )
            ms8 = sm.tile([128, KB], F32, tag="ms8", name="ms8")
            nc.vector.tensor_scalar(
                out=ms8, in0=eb8, scalar1=MAGIC8_HI, scalar2=MAGIC8_LO, op0=A.mult, op1=A.max,
            )
            nc.vector.tensor_tensor(out=sf, in0=sf, in1=ms8, op=A.add)
            nc.vector.tensor_tensor(out=sf, in0=sf, in1=ms8, op=A.subtract)
            nc.vector.tensor_scalar(out=sf, in0=sf, scalar1=0.5, scalar2=None, op0=A.mult)

            # final quantization with the chosen scale (signed)
            B2f = big.tile([128, K], F32, tag="sA", name="B2f")
            B2f3 = B2f.rearrange("p (b e) -> p b e", e=32)
            bc_b = best_c.unsqueeze(2).broadcast_to([128, KB, 32])
            nc.vector.tensor_tensor(out=B2f3, in0=b2s.rearrange("p (b e) -> p b e", e=32), in1=bc_b, op=A.mult)
            if prefetch_next:
                # next group's bmax/b2s/a2s + scalar clip chain, hidden
                # under this group's final pass
                emit_prep_rest(g + 1)
            ttaf = big.tile([128, K], F32, tag="sB", name="ttaf")
            nc.vector.tensor_scalar(
                out=ttaf.bitcast(I32), in0=B2f.bitcast(I32),
                scalar1=EXP_MASK, scalar2=None, op0=A.bitwise_and,
            )
            msvf = big.tile([128, K], F32, tag="sD", name="msvf")
            nc.vector.tensor_scalar(
                out=msvf, in0=ttaf, scalar1=MAGIC / 2.0, scalar2=MAGIC,
                op0=A.mult, op1=A.max,
            )
            rf = big.tile([128, K], F32, tag="sC", name="rf")
            nc.vector.tensor_tensor(out=rf, in0=B2f, in1=msvf, op=A.add)
            qf = big.tile([128, K], F32, tag="sE", name="qf")
            nc.vector.scalar_tensor_tensor(
                out=qf, in0=msvf, scalar=-1.0, in1=rf, op0=A.mult, op1=A.add,
            )
            qc = big.tile([128, K], F32, tag="sB", name="qc")
            nc.vector.tensor_scalar(
                out=qc, in0=qf, scalar1=12.0, scalar2=-12.0, op0=A.min, op1=A.max,
            )
            wdq = big.tile([128, K], BF16, tag="wdq", name="wdq")
            sf_b = sf.unsqueeze(2).broadcast_to([128, KB, 32])
            nc.vector.tensor_tensor(
                out=wdq.rearrange("p (b e) -> p b e", e=32),
                in0=qc.rearrange("p (b e) -> p b e", e=32),
                in1=sf_b, op=A.mult,
            )

            # transpose into wdqT[:, kc, g*128:(g+1)*128]
            for kc in range(KC):
                pt = psum.tile([128, 128], BF16, tag="ptr", bufs=2, name="pt")
                nc.tensor.transpose(pt, wdq[:, kc * 128 : (kc + 1) * 128], ident)
                nc.scalar.copy(out=wdqT[:, kc, g * 128 : (g + 1) * 128], in_=pt)

        def emit_quant_group(g):
            emit_search(g)
            if g + 1 < NG:
                emit_wt_load(g + 1)
            emit_final(g, prefetch_next=(g + 1 < NG))

        # Matmul runs over GROUP-PAIRS: each xt tile feeds 4 MMs (2 groups
        # x 2 m-groups), halving DMA bytes per MM vs per-group passes.
        # Pair {0,1} hides under quant of groups 2,3; pair {2,3} is the tail.
        pair_psums = {}

        def emit_pair_mms(pair, rings, inline_drain=False):
            g0 = 2 * pair
            pair_psums[pair] = []
            for mc in range(M // 1024):
                psums = [
                    psum.tile([128, 512], F32, tag=f"pp{j}", name=f"pp{j}")
                    for j in range(4)
                ]
                pair_psums[pair].append(psums)
                for kc in range(KC):
                    xt = xtp.tile([128, 1024], BF16, tag="xt", name="xt")
                    rings[(mc * KC + kc) % len(rings)].dma_start(
                        out=xt,
                        in_=xT[kc * 128 : (kc + 1) * 128,
                               mc * 1024 : (mc + 1) * 1024],
                    )
                    for gi in range(2):
                        for j in range(2):
                            nc.tensor.matmul(
                                psums[gi * 2 + j],
                                lhsT=wdqT[:, kc, (g0 + gi) * 128 : (g0 + gi + 1) * 128],
                                rhs=xt[:, j * 512 : (j + 1) * 512],
                                start=(kc == 0),
                                stop=(kc == KC - 1),
                            )
                if inline_drain:
                    emit_chunk_drain(g0, mc, psums)

        def emit_chunk_drain(g0, mc, psums):
            for gi in range(2):
                for j in range(2):
                    ysb = ytp.tile([128, 512], F32, tag="ysb", name="ysb")
                    nc.scalar.add(out=ysb, in_=psums[gi * 2 + j], add=bias_sb[g0 + gi])
                    g, mg = g0 + gi, mc * 2 + j
                    nc.sync.dma_start(
                        out=yT[g * 128 : (g + 1) * 128, mg * 512 : (mg + 1) * 512],
                        in_=ysb,
                    )

        def emit_pair_tail(pair):
            # bias-add drain on the DVE: emitted after all quant, so it
            # executes during the tail matmul window where DVE is idle -
            # and the scheduler cannot hoist it into the Scalar engine's
            # quant-critical Au/Ac chain.
            g0 = 2 * pair
            for mc, psums in enumerate(pair_psums[pair]):
                for gi in range(2):
                    for j in range(2):
                        ysb = ytp.tile([128, 512], F32, tag="ysb", name="ysb")
                        nc.vector.tensor_scalar(
                            out=ysb, in0=psums[gi * 2 + j],
                            scalar1=bias_sb[g0 + gi], scalar2=None, op0=A.add,
                        )
                        g, mg = g0 + gi, mc * 2 + j
                        nc.sync.dma_start(
                            out=yT[g * 128 : (g + 1) * 128, mg * 512 : (mg + 1) * 512],
                            in_=ysb,
                        )

        cur[("wt", 0)] = first_wt
        emit_prep_rest(0)
        emit_quant_group(0)
        emit_quant_group(1)
        emit_pair_mms(0, [nc.sync, nc.gpsimd])
        emit_quant_group(2)
        emit_quant_group(3)
        emit_pair_tail(0)
        emit_pair_mms(1, [nc.sync, nc.gpsimd], inline_drain=True)

    nc.compile()
    return nc


_NC_CACHE = None


def _in_maps(x, weight, bias):
    import ml_dtypes

    x = np.ascontiguousarray(x, dtype=np.float32)
    weight = np.ascontiguousarray(weight, dtype=np.float32)
    bias = np.ascontiguousarray(bias, dtype=np.float32)
    xT = np.ascontiguousarray(x.T).astype(ml_dtypes.bfloat16)
    in_maps = []
    for c in range(NCORES):
        in_maps.append(
            {
                "xT": xT,
                "w": weight[c * NSH : (c + 1) * NSH],
                "bias": bias[c * NSH : (c + 1) * NSH].reshape(NSH, 1),
            }
        )
    return in_maps


def kernel(x: np.ndarray, weight: np.ndarray, bias: np.ndarray) -> np.ndarray:
    global _NC_CACHE
    if _NC_CACHE is None:
        _NC_CACHE = build_nc()
    nc = _NC_CACHE
    res = run_bass_kernel_spmd(nc, _in_maps(x, weight, bias), list(range(NCORES)))
    yT = np.concatenate([res.results[c]["yT"] for c in range(NCORES)], axis=0)
    return np.ascontiguousarray(yT.T)


def profile_once(x, weight, bias):
    global _NC_CACHE
    if _NC_CACHE is None:
        _NC_CACHE = build_nc()
    nc = _NC_CACHE
    res = run_bass_kernel_spmd(
        nc, _in_maps(x, weight, bias), list(range(NCORES)),
        trace=True, tmpdir="/tmp/nvfp4_trace",
    )
    print("exec_time_ns:", res.exec_time_ns, "mean:", res.mean_exec_time_ns,
          "max_core:", res.max_exec_time_core_id)
    return res.exec_time_ns


# revision 25
# speedup vs baseline: 1.5201x; 1.0034x over previous
"""NVFP4 fake-quantized linear layer on 8 Trainium2 NeuronCores.

Computes: y = x @ dequant(nvfp4_quantize(weight)).T + bias
  x [8192, 4096] f32, weight [4096, 4096] f32, bias [4096] f32.

Strategy (tensor-parallel, row-wise weight sharding, 512 rows/core):
  - x is transposed and cast to bf16 on the host (layout/precision prep);
    every core receives the full xT [K, M] so the matmul phase streams
    natural [128k, m] tiles with plain DMA - no transpose engine, no
    AllGather on the critical path.
  - Quantization (per-(row, 32-block) MSE scale search, bit-faithful fp32
    incl. fp8-e4m3 scale rounding emulated in fp32) is software-pipelined
    across engines with zero-stall buffer rotation:
      ScalarE (ratio i+1): Au = Relu(12 - c*a2s); Ac = 12 - Au  (clip)
      DVE     (ratio i):   tta = Ac & EXP_MASK (exp2 floor bits)
                           msv = max(tta*MAGIC/2, MAGIC)  (fused clamp:
                                 max(2^e,2)*M/2 == max(2^e*M/2, M))
                           r = Ac + msv ; q = r - msv  (magic RNE round)
                           d = a2s*c - q  (unclipped error -> wt buffer)
      ScalarE: dsq = Square(ratio*d)  (ratio^2 MSE weighting folded in)
      DVE:     e = per-32-block reduce(dsq); argmin bookkeeping
    TensorE transposes the dequantized weights into wdqT bf16.
  - Each group's prep (w load, bmax, b2s, |.|, first clip) is emitted
    under the previous group's final pass, removing the group-boundary
    serial chain from the critical path.
  - Matmul runs over GROUP-PAIRS so each xt tile feeds 4 MMs (2 groups x
    2 m-groups), halving DMA bytes per matmul: pair {0,1} executes hidden
    under quantization of groups 2,3 (xt streamed on two DMA rings);
    pair {2,3} is the tail with inline per-chunk psum drains. Bias is
    added during the PSUM->SBUF drain (DVE for pair 0 - it lands in the
    tail window and keeps ScalarE's quant chain clean; ScalarE for pair
    1). Each core writes yT [512, 8192]; the host concatenates+transposes.
"""

import sys

sys.path.insert(0, "/opt/trn_rl_repo")

from contextlib import ExitStack

import numpy as np

import concourse.bass as bass
import concourse.bacc as bacc
import concourse.tile as tile
from concourse import mybir
from concourse.bass_utils import run_bass_kernel_spmd

A = mybir.AluOpType
AF = mybir.ActivationFunctionType
F32 = mybir.dt.float32
BF16 = mybir.dt.bfloat16
I32 = mybir.dt.int32

NCORES = 8
M, K, N = 8192, 4096, 4096
NSH = N // NCORES          # 512 weight rows per core
NG = NSH // 128            # 4 row groups per core
KC = K // 128              # 32 contraction chunks
KB = K // 32               # 128 blocks per weight row
MG = M // 512              # 16 output m-groups
MCH = 4                    # m-groups per psum chunk (4 banks live)

RATIOS = [float(r) for r in np.linspace(0.7, 1.0, 10)]
MAGIC = 12582912.0         # 1.5 * 2**23 : RNE-to-integer magic constant
INF = float("inf")
EXP_MASK = 0x7F800000      # fp32 exponent field mask
ABS_MASK = 0x7FFFFFFF      # clears the sign bit
TWO_BITS = 0x40000000      # bits of 2.0f; int max == float max for positives
# fp8 e4m3 rounding grid: step = max(2^-9, exp2floor(x) * 2^-3)
MAGIC8_HI = MAGIC / 8.0
MAGIC8_LO = MAGIC / 512.0


def build_nc() -> bass.Bass:
    nc = bacc.Bacc("TRN2", num_devices=NCORES)

    xT = nc.declare_dram_parameter("xT", [K, M], BF16, isOutput=False)
    w = nc.declare_dram_parameter("w", [NSH, K], F32, isOutput=False)
    bias = nc.declare_dram_parameter("bias", [NSH, 1], F32, isOutput=False)
    yT = nc.declare_dram_parameter("yT", [NSH, M], F32, isOutput=True)

    with tile.TileContext(nc) as tc, ExitStack() as ctx:
        big = ctx.enter_context(tc.tile_pool(name="big", bufs=1))
        sm = ctx.enter_context(tc.tile_pool(name="small", bufs=1))
        wtp = ctx.enter_context(tc.tile_pool(name="wtp", bufs=1))
        psum = ctx.enter_context(tc.tile_pool(name="psum", bufs=1, space="PSUM"))
        xtp = ctx.enter_context(tc.tile_pool(name="xtp", bufs=8))
        ytp = ctx.enter_context(tc.tile_pool(name="ytp", bufs=2))

        # persistent w_dq^T, bf16 [128 k-partitions, 32 k-chunks, 512 n]
        wdqT = big.tile([128, KC, NSH], BF16, tag="wdqT", name="wdqT")

        ident = sm.tile([128, 128], BF16, tag="ident", name="ident")
        from concourse.masks import make_identity

        make_identity(nc, ident)

        first_wt = wtp.tile([128, K], F32, tag="wt", name="wt")
        nc.scalar.dma_start(out=first_wt, in_=w[0:128, :])

        bias_sb = []
        for g in range(NG):
            bsl = sm.tile([128, 1], F32, tag=f"bias{g}", name=f"bias{g}")
            nc.scalar.dma_start(out=bsl, in_=bias[g * 128 : (g + 1) * 128, :])
            bias_sb.append(bsl)

        twelve = sm.tile([128, 1], F32, tag="twelve", name="twelve")
        nc.vector.memset(twelve, 12.0)

        # hoisted per-ratio constant tiles for the argmin bookkeeping
        cconst, rconst = [], []
        for i, ratio in enumerate(RATIOS):
            ct = sm.tile([128, KB], F32, tag=f"cc{i}", name=f"cc{i}")
            nc.vector.memset(ct, float(np.float32(1.0) / np.float32(ratio)))
            cconst.append(ct)
            rt = sm.tile([128, KB], F32, tag=f"rc{i}", name=f"rc{i}")
            nc.vector.memset(rt, float(np.float32(ratio)))
            rconst.append(rt)

        # Per-group quant state; bmax alternates tags so group g's sf can
        # still read it after group g+1's prep has started.
        cur = {}

        def emit_wt_load(g):
            # wt shares its buffer with the ratio loop's d tile; emitting
            # the load right after the last d keeps WAR order correct and
            # gives the DMA the whole final pass to complete.
            wt = wtp.tile([128, K], F32, tag="wt", name="wt")
            nc.scalar.dma_start(out=wt, in_=w[g * 128 : (g + 1) * 128, :])
            cur[("wt", g)] = wt

        def emit_prep_rest(g):
            # bmax/inv/b2s/a2s for group g - emitted under group g-1's
            # final pass so the scalar Abs->Au->Ac chain is off the
            # group-boundary critical path.
            wt = cur.pop(("wt", g))
            wt3 = wt.rearrange("p (b e) -> p b e", e=32)
            bmax = sm.tile([128, KB], F32, tag=f"bmax{g % 2}", name="bmax")
            nc.vector.tensor_reduce(
                out=bmax, in_=wt3, axis=mybir.AxisListType.X, op=A.max,
                apply_absolute_value=True,
            )
            nc.vector.tensor_scalar(out=bmax, in0=bmax, scalar1=1e-12, scalar2=None, op0=A.max)
            inv = sm.tile([128, KB], F32, tag="inv", name="inv")
            nc.vector.reciprocal(out=inv, in_=bmax)
            b2s = big.tile([128, K], F32, tag="b2s", name="b2s")
            b2s3 = b2s.rearrange("p (b e) -> p b e", e=32)
            inv_b = inv.unsqueeze(2).broadcast_to([128, KB, 32])
            nc.vector.scalar_tensor_tensor(
                out=b2s3, in0=wt3, scalar=12.0, in1=inv_b, op0=A.mult, op1=A.mult,
            )
            a2s = big.tile([128, K], F32, tag="a2s", name="a2s")
            nc.scalar.activation(out=a2s, in_=b2s, func=AF.Abs)
            cur["bmax"], cur["b2s"], cur["a2s"] = bmax, b2s, a2s
            # pre-warm ratio 0's clip on ScalarE
            cur["Ac0"] = emit_clip(0)

        def emit_clip(i):
            # Ac = min(a2s*c_i, 12) via 12 - Relu(12 - c_i*a2s) on ScalarE
            c = float(np.float32(1.0) / np.float32(RATIOS[i]))
            Au = big.tile([128, K], F32, tag="sE", name="Au")
            nc.scalar.activation(
                out=Au, in_=cur["a2s"], func=AF.Relu, scale=-c, bias=twelve,
            )
            Ac = big.tile([128, K], F32, tag="sA", name="Ac")
            nc.scalar.activation(
                out=Ac, in_=Au, func=AF.Identity, scale=-1.0, bias=twelve,
            )
            return Ac

        def emit_mask(Ac):
            # tta = exp2floor-bits(Ac);  msv = max(tta*MAGIC/2, MAGIC)
            # (identity: max(2^e, 2)*M/2 == max(2^e*M/2, M))
            tta = big.tile([128, K], F32, tag="sB", name="tta")
            nc.vector.tensor_scalar(
                out=tta.bitcast(I32), in0=Ac.bitcast(I32),
                scalar1=EXP_MASK, scalar2=None, op0=A.bitwise_and,
            )
            msv = big.tile([128, K], F32, tag="sD", name="msv")
            nc.vector.tensor_scalar(
                out=msv, in0=tta, scalar1=MAGIC / 2.0, scalar2=MAGIC,
                op0=A.mult, op1=A.max,
            )
            return msv

        def emit_search(g):
            a2s = cur["a2s"]
            best_e = sm.tile([128, KB], F32, tag="best_e", name="best_e")
            nc.vector.memset(best_e, INF)
            best_c = sm.tile([128, KB], F32, tag="best_c", name="best_c")
            nc.vector.memset(best_c, 0.0)
            best_r = sm.tile([128, KB], F32, tag="best_r", name="best_r")
            nc.vector.memset(best_r, 1.0)
            cur["best_c"], cur["best_r"] = best_c, best_r

            # Software-pipelined MSE search: ScalarE computes ratio i+1's
            # clipped operand while DVE rounds ratio i; DVE hides the
            # Square latency under ratio i+1's tta/msv. Zero-stall.
            Ac = cur.pop("Ac0")
            msv = emit_mask(Ac)
            for i, ratio in enumerate(RATIOS):
                c = float(np.float32(1.0) / np.float32(ratio))
                # r = Ac + msv ; q = r - msv  (RNE onto the e2m1 grid)
                r_ = big.tile([128, K], F32, tag="sC", name="r")
                nc.vector.tensor_tensor(out=r_, in0=Ac, in1=msv, op=A.add)
                q_ = big.tile([128, K], F32, tag="sB", name="q")
                nc.vector.scalar_tensor_tensor(
                    out=q_, in0=msv, scalar=-1.0, in1=r_, op0=A.mult, op1=A.add,
                )
                if i + 1 < len(RATIOS):
                    Ac = emit_clip(i + 1)
                # d = a2s*c - q (unclipped error, matches reference MSE);
                # lives in the idle wt buffer
                d_ = wtp.tile([128, K], F32, tag="wt", name="d")
                nc.vector.scalar_tensor_tensor(
                    out=d_, in0=a2s, scalar=c, in1=q_, op0=A.mult, op1=A.subtract,
                )
                if i + 1 < len(RATIOS):
                    msv = emit_mask(Ac)
                # dsq = (ratio*d)^2 : folds the ratio^2 MSE weighting in
                dsq = big.tile([128, K], F32, tag="sC", name="dsq")
                nc.scalar.activation(
                    out=dsq, in_=d_, func=AF.Square, scale=float(np.float32(ratio)),
                )
                e_ = sm.tile([128, KB], F32, tag="e", name="e")
                nc.vector.tensor_reduce(
                    out=e_, in_=dsq.rearrange("p (b e) -> p b e", e=32),
                    axis=mybir.AxisListType.X, op=A.add,
                )
                mask = sm.tile([128, KB], I32, tag="mask", name="mask")
                nc.vector.tensor_tensor(out=mask, in0=e_, in1=best_e, op=A.is_lt)
                nc.vector.tensor_tensor(out=best_e, in0=e_, in1=best_e, op=A.min)
                nc.vector.copy_predicated(out=best_c, mask=mask, data=cconst[i])
                nc.vector.copy_predicated(out=best_r, mask=mask, data=rconst[i])

        def emit_final(g, prefetch_next):
            bmax, b2s = cur["bmax"], cur["b2s"]
            best_c, best_r = cur["best_c"], cur["best_r"]
            # scale factor sf = bmax * best_r / 6, rounded to fp8 e4m3 (RNE,
            # subnormal-aware) emulated in fp32, then halved (q = q2/2).
            sf = sm.tile([128, KB], F32, tag="sf", name="sf")
            nc.vector.scalar_tensor_tensor(
                out=sf, in0=bmax, scalar=1.0 / 6.0, in1=best_r, op0=A.mult, op1=A.mult,
            )
            eb8 = sm.tile([128, KB], F32, tag="eb8", name="eb8")
            nc.vector.tensor_scalar(
                out=eb8.bitcast(I32), in0=sf.bitcast(I32),
                scalar1=EXP_MASK, scalar2=None, op0=A.bitwise_and,
            )
            ms8 = sm.tile([128, KB], F32, tag="ms8", name="ms8")
            nc.vector.tensor_scalar(
                out=ms8, in0=eb8, scalar1=MAGIC8_HI, scalar2=MAGIC8_LO, op0=A.mult, op1=A.max,
            )
            nc.vector.tensor_tensor(out=sf, in0=sf, in1=ms8, op=A.add)
            nc.vector.tensor_tensor(out=sf, in0=sf, in1=ms8, op=A.subtract)
            nc.vector.tensor_scalar(out=sf, in0=sf, scalar1=0.5, scalar2=None, op0=A.mult)

            # final quantization with the chosen scale (signed)
            B2f = big.tile([128, K], F32, tag="sA", name="B2f")
            B2f3 = B2f.rearrange("p (b e) -> p b e", e=32)
            bc_b = best_c.unsqueeze(2).broadcast_to([128, KB, 32])
            nc.vector.tensor_tensor(out=B2f3, in0=b2s.rearrange("p (b e) -> p b e", e=32), in1=bc_b, op=A.mult)
            if prefetch_next:
                # next group's bmax/b2s/a2s + scalar clip chain, hidden
                # under this group's final pass
                emit_prep_rest(g + 1)
            ttaf = big.tile([128, K], F32, tag="sB", name="ttaf")
            nc.vector.tensor_scalar(
                out=ttaf.bitcast(I32), in0=B2f.bitcast(I32),
                scalar1=EXP_MASK, scalar2=None, op0=A.bitwise_and,
            )
            msvf = big.tile([128, K], F32, tag="sD", name="msvf")
            nc.vector.tensor_scalar(
                out=msvf, in0=ttaf, scalar1=MAGIC / 2.0, scalar2=MAGIC,
                op0=A.mult, op1=A.max,
            )
            rf = big.tile([128, K], F32, tag="sC", name="rf")
            nc.vector.tensor_tensor(out=rf, in0=B2f, in1=msvf, op=A.add)
            qf = big.tile([128, K], F32, tag="sE", name="qf")
            nc.vector.scalar_tensor_tensor(
                out=qf, in0=msvf, scalar=-1.0, in1=rf, op0=A.mult, op1=A.add,
            )
            qc = big.tile([128, K], F32, tag="sB", name="qc")
            nc.vector.tensor_scalar(
                out=qc, in0=qf, scalar1=12.0, scalar2=-12.0, op0=A.min, op1=A.max,
            )
            wdq = big.tile([128, K], BF16, tag="wdq", name="wdq")
            sf_b = sf.unsqueeze(2).broadcast_to([128, KB, 32])
            nc.vector.tensor_tensor(
                out=wdq.rearrange("p (b e) -> p b e", e=32),
                in0=qc.rearrange("p (b e) -> p b e", e=32),
                in1=sf_b, op=A.mult,
            )

            # transpose into wdqT[:, kc, g*128:(g+1)*128]
            for kc in range(KC):
                pt = psum.tile([128, 128], BF16, tag="ptr", bufs=2, name="pt")
                nc.tensor.transpose(pt, wdq[:, kc * 128 : (kc + 1) * 128], ident)
                nc.scalar.copy(out=wdqT[:, kc, g * 128 : (g + 1) * 128], in_=pt)

        def emit_quant_group(g):
            emit_search(g)
            if g + 1 < NG:
                emit_wt_load(g + 1)
            emit_final(g, prefetch_next=(g + 1 < NG))

        # Matmul runs over GROUP-PAIRS: each xt tile feeds 4 MMs (2 groups
        # x 2 m-groups), halving DMA bytes per MM vs per-group passes.
        # Pair {0,1} hides under quant of groups 2,3; pair {2,3} is the tail.
        pair_psums = {}

        def emit_pair_mms(pair, rings, inline_drain=False):
            g0 = 2 * pair
            pair_psums[pair] = []
            for mc in range(M // 1024):
                psums = [
                    psum.tile([128, 512], F32, tag=f"pp{j}", name=f"pp{j}")
                    for j in range(4)
                ]
                pair_psums[pair].append(psums)
                for kc in range(KC):
                    xt = xtp.tile([128, 1024], BF16, tag="xt", name="xt")
                    rings[(mc * KC + kc) % len(rings)].dma_start(
                        out=xt,
                        in_=xT[kc * 128 : (kc + 1) * 128,
                               mc * 1024 : (mc + 1) * 1024],
                    )
                    for gi in range(2):
                        for j in range(2):
                            nc.tensor.matmul(
                                psums[gi * 2 + j],
                                lhsT=wdqT[:, kc, (g0 + gi) * 128 : (g0 + gi + 1) * 128],
                                rhs=xt[:, j * 512 : (j + 1) * 512],
                                start=(kc == 0),
                                stop=(kc == KC - 1),
                            )
                if inline_drain:
                    emit_chunk_drain(g0, mc, psums)

        def emit_chunk_drain(g0, mc, psums):
            for gi in range(2):
                for j in range(2):
                    ysb = ytp.tile([128, 512], F32, tag="ysb", name="ysb")
                    nc.scalar.add(out=ysb, in_=psums[gi * 2 + j], add=bias_sb[g0 + gi])
                    g, mg = g0 + gi, mc * 2 + j
                    nc.sync.dma_start(
                        out=yT[g * 128 : (g + 1) * 128, mg * 512 : (mg + 1) * 512],
                        in_=ysb,
                    )

        def emit_pair_tail(pair):
            # bias-add drain on the DVE: emitted after all quant, so it
            # executes during the tail matmul window where DVE is idle -
            # and the scheduler cannot hoist it into the Scalar engine's
            # quant-critical Au/Ac chain.
            g0 = 2 * pair
            for mc, psums in enumerate(pair_psums[pair]):
                for gi in range(2):
                    for j in range(2):
                        ysb = ytp.tile([128, 512], F32, tag="ysb", name="ysb")
                        nc.vector.tensor_scalar(
                            out=ysb, in0=psums[gi * 2 + j],
                            scalar1=bias_sb[g0 + gi], scalar2=None, op0=A.add,
                        )
                        g, mg = g0 + gi, mc * 2 + j
                        nc.sync.dma_start(
                            out=yT[g * 128 : (g + 1) * 128, mg * 512 : (mg + 1) * 512],
                            in_=ysb,
                        )

        cur[("wt", 0)] = first_wt
        emit_prep_rest(0)
        emit_quant_group(0)
        emit_quant_group(1)
        emit_pair_mms(0, [nc.sync, nc.gpsimd])
        emit_quant_group(2)
        emit_quant_group(3)
        emit_pair_tail(0)
        emit_pair_mms(1, [nc.sync, nc.gpsimd], inline_drain=True)

    nc.compile()
    return nc


_NC_CACHE = None


def _in_maps(x, weight, bias):
    import ml_dtypes

    x = np.ascontiguousarray(x, dtype=np.float32)
    weight = np.ascontiguousarray(weight, dtype=np.float32)
    bias = np.ascontiguousarray(bias, dtype=np.float32)
    xT = np.ascontiguousarray(x.T).astype(ml_dtypes.bfloat16)
    in_maps = []
    for c in range(NCORES):
        in_maps.append(
            {
                "xT": xT,
                "w": weight[c * NSH : (c + 1) * NSH],
                "bias": bias[c * NSH : (c + 1) * NSH].reshape(NSH, 1),
            }
        )
    return in_maps


def kernel(x: np.ndarray, weight: np.ndarray, bias: np.ndarray) -> np.ndarray:
    global _NC_CACHE
    if _NC_CACHE is None:
        _NC_CACHE = build_nc()
    nc = _NC_CACHE
    res = run_bass_kernel_spmd(nc, _in_maps(x, weight, bias), list(range(NCORES)))
    yT = np.concatenate([res.results[c]["yT"] for c in range(NCORES)], axis=0)
    return np.ascontiguousarray(yT.T)


def profile_once(x, weight, bias):
    global _NC_CACHE
    if _NC_CACHE is None:
        _NC_CACHE = build_nc()
    nc = _NC_CACHE
    res = run_bass_kernel_spmd(
        nc, _in_maps(x, weight, bias), list(range(NCORES)),
        trace=True, tmpdir="/tmp/nvfp4_trace",
    )
    print("exec_time_ns:", res.exec_time_ns, "mean:", res.mean_exec_time_ns,
          "max_core:", res.max_exec_time_core_id)
    return res.exec_time_ns


# revision 26
# speedup vs baseline: 1.5203x; 1.0001x over previous
"""NVFP4 fake-quantized linear layer on 8 Trainium2 NeuronCores.

Computes: y = x @ dequant(nvfp4_quantize(weight)).T + bias
  x [8192, 4096] f32, weight [4096, 4096] f32, bias [4096] f32.

Strategy (tensor-parallel, row-wise weight sharding, 512 rows/core):
  - x is transposed and cast to bf16 on the host (layout/precision prep);
    every core receives the full xT [K, M] so the matmul phase streams
    natural [128k, m] tiles with plain DMA - no transpose engine, no
    AllGather on the critical path.
  - Quantization (per-(row, 32-block) MSE scale search, bit-faithful fp32
    incl. fp8-e4m3 scale rounding emulated in fp32) is software-pipelined
    across engines with zero-stall buffer rotation:
      ScalarE (ratio i+1): Au = Relu(12 - c*a2s); Ac = 12 - Au  (clip)
      DVE     (ratio i):   tta = Ac & EXP_MASK (exp2 floor bits)
                           msv = max(tta*MAGIC/2, MAGIC)  (fused clamp:
                                 max(2^e,2)*M/2 == max(2^e*M/2, M))
                           r = Ac + msv ; q = r - msv  (magic RNE round)
                           d = a2s*c - q  (unclipped error -> wt buffer)
      ScalarE: dsq = Square(ratio*d)  (ratio^2 MSE weighting folded in)
      DVE:     e = per-32-block reduce(dsq); argmin bookkeeping
    TensorE transposes the dequantized weights into wdqT bf16.
  - Each group's prep (w load, bmax, b2s, |.|, first clip) is emitted
    under the previous group's final pass, removing the group-boundary
    serial chain from the critical path.
  - Matmul runs over GROUP-PAIRS so each xt tile feeds 4 MMs (2 groups x
    2 m-groups), halving DMA bytes per matmul: pair {0,1} executes hidden
    under quantization of groups 2,3 (xt streamed on two DMA rings);
    pair {2,3} is the tail with inline per-chunk psum drains. Bias is
    added during the PSUM->SBUF drain (DVE for pair 0 - it lands in the
    tail window and keeps ScalarE's quant chain clean; ScalarE for pair
    1). Each core writes yT [512, 8192]; the host concatenates+transposes.
"""

import sys

sys.path.insert(0, "/opt/trn_rl_repo")

from contextlib import ExitStack

import numpy as np

import concourse.bass as bass
import concourse.bacc as bacc
import concourse.tile as tile
from concourse import mybir
from concourse.bass_utils import run_bass_kernel_spmd

A = mybir.AluOpType
AF = mybir.ActivationFunctionType
F32 = mybir.dt.float32
BF16 = mybir.dt.bfloat16
I32 = mybir.dt.int32

NCORES = 8
M, K, N = 8192, 4096, 4096
NSH = N // NCORES          # 512 weight rows per core
NG = NSH // 128            # 4 row groups per core
KC = K // 128              # 32 contraction chunks
KB = K // 32               # 128 blocks per weight row
MG = M // 512              # 16 output m-groups
MCH = 4                    # m-groups per psum chunk (4 banks live)

RATIOS = [float(r) for r in np.linspace(0.7, 1.0, 10)]
MAGIC = 12582912.0         # 1.5 * 2**23 : RNE-to-integer magic constant
INF = float("inf")
EXP_MASK = 0x7F800000      # fp32 exponent field mask
ABS_MASK = 0x7FFFFFFF      # clears the sign bit
TWO_BITS = 0x40000000      # bits of 2.0f; int max == float max for positives
# fp8 e4m3 rounding grid: step = max(2^-9, exp2floor(x) * 2^-3)
MAGIC8_HI = MAGIC / 8.0
MAGIC8_LO = MAGIC / 512.0


def build_nc() -> bass.Bass:
    nc = bacc.Bacc("TRN2", num_devices=NCORES)

    xT = nc.declare_dram_parameter("xT", [K, M], BF16, isOutput=False)
    w = nc.declare_dram_parameter("w", [NSH, K], F32, isOutput=False)
    bias = nc.declare_dram_parameter("bias", [NSH, 1], F32, isOutput=False)
    yT = nc.declare_dram_parameter("yT", [NSH, M], F32, isOutput=True)

    with tile.TileContext(nc) as tc, ExitStack() as ctx:
        big = ctx.enter_context(tc.tile_pool(name="big", bufs=1))
        sm = ctx.enter_context(tc.tile_pool(name="small", bufs=1))
        wtp = ctx.enter_context(tc.tile_pool(name="wtp", bufs=1))
        psum = ctx.enter_context(tc.tile_pool(name="psum", bufs=1, space="PSUM"))
        xtp = ctx.enter_context(tc.tile_pool(name="xtp", bufs=8))
        ytp = ctx.enter_context(tc.tile_pool(name="ytp", bufs=2))

        # persistent w_dq^T, bf16 [128 k-partitions, 32 k-chunks, 512 n]
        wdqT = big.tile([128, KC, NSH], BF16, tag="wdqT", name="wdqT")

        ident = sm.tile([128, 128], BF16, tag="ident", name="ident")
        from concourse.masks import make_identity

        make_identity(nc, ident)

        first_wt = wtp.tile([128, K], F32, tag="wt", name="wt")
        nc.scalar.dma_start(out=first_wt, in_=w[0:128, :])

        bias_sb = []
        for g in range(NG):
            bsl = sm.tile([128, 1], F32, tag=f"bias{g}", name=f"bias{g}")
            nc.scalar.dma_start(out=bsl, in_=bias[g * 128 : (g + 1) * 128, :])
            bias_sb.append(bsl)

        twelve = sm.tile([128, 1], F32, tag="twelve", name="twelve")
        nc.vector.memset(twelve, 12.0)

        # hoisted per-ratio constant tiles for the argmin bookkeeping
        cconst, rconst = [], []
        for i, ratio in enumerate(RATIOS):
            ct = sm.tile([128, KB], F32, tag=f"cc{i}", name=f"cc{i}")
            nc.vector.memset(ct, float(np.float32(1.0) / np.float32(ratio)))
            cconst.append(ct)
            rt = sm.tile([128, KB], F32, tag=f"rc{i}", name=f"rc{i}")
            nc.vector.memset(rt, float(np.float32(ratio)))
            rconst.append(rt)

        # Per-group quant state; bmax alternates tags so group g's sf can
        # still read it after group g+1's prep has started.
        cur = {}

        def emit_wt_load(g):
            # wt shares its buffer with the ratio loop's d tile; emitting
            # the load right after the last d keeps WAR order correct and
            # gives the DMA the whole final pass to complete.
            wt = wtp.tile([128, K], F32, tag="wt", name="wt")
            nc.scalar.dma_start(out=wt, in_=w[g * 128 : (g + 1) * 128, :])
            cur[("wt", g)] = wt

        def emit_prep_rest(g):
            # bmax/inv/b2s/a2s for group g - emitted under group g-1's
            # final pass so the scalar Abs->Au->Ac chain is off the
            # group-boundary critical path.
            wt = cur.pop(("wt", g))
            wt3 = wt.rearrange("p (b e) -> p b e", e=32)
            bmax = sm.tile([128, KB], F32, tag=f"bmax{g % 2}", name="bmax")
            nc.vector.tensor_reduce(
                out=bmax, in_=wt3, axis=mybir.AxisListType.X, op=A.max,
                apply_absolute_value=True,
            )
            nc.vector.tensor_scalar(out=bmax, in0=bmax, scalar1=1e-12, scalar2=None, op0=A.max)
            inv = sm.tile([128, KB], F32, tag="inv", name="inv")
            nc.vector.reciprocal(out=inv, in_=bmax)
            b2s = big.tile([128, K], F32, tag="b2s", name="b2s")
            b2s3 = b2s.rearrange("p (b e) -> p b e", e=32)
            inv_b = inv.unsqueeze(2).broadcast_to([128, KB, 32])
            nc.vector.scalar_tensor_tensor(
                out=b2s3, in0=wt3, scalar=12.0, in1=inv_b, op0=A.mult, op1=A.mult,
            )
            a2s = big.tile([128, K], F32, tag="a2s", name="a2s")
            nc.scalar.activation(out=a2s, in_=b2s, func=AF.Abs)
            cur["bmax"], cur["b2s"], cur["a2s"] = bmax, b2s, a2s
            # pre-warm ratio 0's clip on ScalarE
            cur["Ac0"] = emit_clip(0)

        def emit_clip(i):
            # Ac = min(a2s*c_i, 12) via 12 - Relu(12 - c_i*a2s) on ScalarE
            c = float(np.float32(1.0) / np.float32(RATIOS[i]))
            Au = big.tile([128, K], F32, tag="sE", name="Au")
            nc.scalar.activation(
                out=Au, in_=cur["a2s"], func=AF.Relu, scale=-c, bias=twelve,
            )
            Ac = big.tile([128, K], F32, tag="sA", name="Ac")
            nc.scalar.activation(
                out=Ac, in_=Au, func=AF.Identity, scale=-1.0, bias=twelve,
            )
            return Ac

        def emit_mask(Ac):
            # tta = exp2floor-bits(Ac);  msv = max(tta*MAGIC/2, MAGIC)
            # (identity: max(2^e, 2)*M/2 == max(2^e*M/2, M))
            tta = big.tile([128, K], F32, tag="sB", name="tta")
            nc.vector.tensor_scalar(
                out=tta.bitcast(I32), in0=Ac.bitcast(I32),
                scalar1=EXP_MASK, scalar2=None, op0=A.bitwise_and,
            )
            msv = big.tile([128, K], F32, tag="sD", name="msv")
            nc.vector.tensor_scalar(
                out=msv, in0=tta, scalar1=MAGIC / 2.0, scalar2=MAGIC,
                op0=A.mult, op1=A.max,
            )
            return msv

        def emit_search(g):
            a2s = cur["a2s"]
            best_e = sm.tile([128, KB], F32, tag="best_e", name="best_e")
            nc.vector.memset(best_e, INF)
            best_c = sm.tile([128, KB], F32, tag="best_c", name="best_c")
            nc.vector.memset(best_c, 0.0)
            best_r = sm.tile([128, KB], F32, tag="best_r", name="best_r")
            nc.vector.memset(best_r, 1.0)
            cur["best_c"], cur["best_r"] = best_c, best_r

            # Software-pipelined MSE search: ScalarE computes ratio i+1's
            # clipped operand while DVE rounds ratio i; DVE hides the
            # Square latency under ratio i+1's tta/msv. Zero-stall.
            Ac = cur.pop("Ac0")
            msv = emit_mask(Ac)
            for i, ratio in enumerate(RATIOS):
                c = float(np.float32(1.0) / np.float32(ratio))
                # r = Ac + msv ; q = r - msv  (RNE onto the e2m1 grid)
                r_ = big.tile([128, K], F32, tag="sC", name="r")
                nc.vector.tensor_tensor(out=r_, in0=Ac, in1=msv, op=A.add)
                q_ = big.tile([128, K], F32, tag="sB", name="q")
                nc.vector.scalar_tensor_tensor(
                    out=q_, in0=msv, scalar=-1.0, in1=r_, op0=A.mult, op1=A.add,
                )
                if i + 1 < len(RATIOS):
                    Ac = emit_clip(i + 1)
                # d = a2s*c - q (unclipped error, matches reference MSE);
                # lives in the idle wt buffer
                d_ = wtp.tile([128, K], F32, tag="wt", name="d")
                nc.vector.scalar_tensor_tensor(
                    out=d_, in0=a2s, scalar=c, in1=q_, op0=A.mult, op1=A.subtract,
                )
                if i + 1 < len(RATIOS):
                    msv = emit_mask(Ac)
                # dsq = (ratio*d)^2 : folds the ratio^2 MSE weighting in
                dsq = big.tile([128, K], F32, tag="sC", name="dsq")
                nc.scalar.activation(
                    out=dsq, in_=d_, func=AF.Square, scale=float(np.float32(ratio)),
                )
                e_ = sm.tile([128, KB], F32, tag="e", name="e")
                nc.vector.tensor_reduce(
                    out=e_, in_=dsq.rearrange("p (b e) -> p b e", e=32),
                    axis=mybir.AxisListType.X, op=A.add,
                )
                mask = sm.tile([128, KB], I32, tag="mask", name="mask")
                nc.vector.tensor_tensor(out=mask, in0=e_, in1=best_e, op=A.is_lt)
                nc.vector.tensor_tensor(out=best_e, in0=e_, in1=best_e, op=A.min)
                nc.vector.copy_predicated(out=best_c, mask=mask, data=cconst[i])
                nc.vector.copy_predicated(out=best_r, mask=mask, data=rconst[i])

        def emit_final(g, prefetch_next):
            bmax, b2s = cur["bmax"], cur["b2s"]
            best_c, best_r = cur["best_c"], cur["best_r"]
            # scale factor sf = bmax * best_r / 6, rounded to fp8 e4m3 (RNE,
            # subnormal-aware) emulated in fp32, then halved (q = q2/2).
            sf = sm.tile([128, KB], F32, tag="sf", name="sf")
            nc.vector.scalar_tensor_tensor(
                out=sf, in0=bmax, scalar=1.0 / 6.0, in1=best_r, op0=A.mult, op1=A.mult,
            )
            eb8 = sm.tile([128, KB], F32, tag="eb8", name="eb8")
            nc.vector.tensor_scalar(
                out=eb8.bitcast(I32), in0=sf.bitcast(I32),
                scalar1=EXP_MASK, scalar2=None, op0=A.bitwise_and,
            )
            ms8 = sm.tile([128, KB], F32, tag="ms8", name="ms8")
            nc.vector.tensor_scalar(
                out=ms8, in0=eb8, scalar1=MAGIC8_HI, scalar2=MAGIC8_LO, op0=A.mult, op1=A.max,
            )
            nc.vector.tensor_tensor(out=sf, in0=sf, in1=ms8, op=A.add)
            nc.vector.tensor_tensor(out=sf, in0=sf, in1=ms8, op=A.subtract)
            nc.vector.tensor_scalar(out=sf, in0=sf, scalar1=0.5, scalar2=None, op0=A.mult)

            # final quantization with the chosen scale (signed)
            B2f = big.tile([128, K], F32, tag="sA", name="B2f")
            B2f3 = B2f.rearrange("p (b e) -> p b e", e=32)
            bc_b = best_c.unsqueeze(2).broadcast_to([128, KB, 32])
            nc.vector.tensor_tensor(out=B2f3, in0=b2s.rearrange("p (b e) -> p b e", e=32), in1=bc_b, op=A.mult)
            if prefetch_next:
                # next group's bmax/b2s/a2s + scalar clip chain, hidden
                # under this group's final pass
                emit_prep_rest(g + 1)
            ttaf = big.tile([128, K], F32, tag="sB", name="ttaf")
            nc.vector.tensor_scalar(
                out=ttaf.bitcast(I32), in0=B2f.bitcast(I32),
                scalar1=EXP_MASK, scalar2=None, op0=A.bitwise_and,
            )
            msvf = big.tile([128, K], F32, tag="sD", name="msvf")
            nc.vector.tensor_scalar(
                out=msvf, in0=ttaf, scalar1=MAGIC / 2.0, scalar2=MAGIC,
                op0=A.mult, op1=A.max,
            )
            rf = big.tile([128, K], F32, tag="sC", name="rf")
            nc.vector.tensor_tensor(out=rf, in0=B2f, in1=msvf, op=A.add)
            qf = big.tile([128, K], F32, tag="sE", name="qf")
            nc.vector.scalar_tensor_tensor(
                out=qf, in0=msvf, scalar=-1.0, in1=rf, op0=A.mult, op1=A.add,
            )
            qc = big.tile([128, K], F32, tag="sB", name="qc")
            nc.vector.tensor_scalar(
                out=qc, in0=qf, scalar1=12.0, scalar2=-12.0, op0=A.min, op1=A.max,
            )
            wdq = big.tile([128, K], BF16, tag="wdq", name="wdq")
            sf_b = sf.unsqueeze(2).broadcast_to([128, KB, 32])
            nc.vector.tensor_tensor(
                out=wdq.rearrange("p (b e) -> p b e", e=32),
                in0=qc.rearrange("p (b e) -> p b e", e=32),
                in1=sf_b, op=A.mult,
            )

            # transpose into wdqT[:, kc, g*128:(g+1)*128]
            for kc in range(KC):
                pt = psum.tile([128, 128], BF16, tag="ptr", bufs=2, name="pt")
                nc.tensor.transpose(pt, wdq[:, kc * 128 : (kc + 1) * 128], ident)
                nc.scalar.copy(out=wdqT[:, kc, g * 128 : (g + 1) * 128], in_=pt)

        def emit_quant_group(g):
            emit_search(g)
            if g + 1 < NG:
                emit_wt_load(g + 1)
            emit_final(g, prefetch_next=(g + 1 < NG))

        # Matmul runs over GROUP-PAIRS: each xt tile feeds 4 MMs (2 groups
        # x 2 m-groups), halving DMA bytes per MM vs per-group passes.
        # Pair {0,1} hides under quant of groups 2,3; pair {2,3} is the tail.
        pair_psums = {}

        def emit_pair_mms(pair, rings, inline_drain=False):
            g0 = 2 * pair
            pair_psums[pair] = []
            for mc in range(M // 1024):
                psums = [
                    psum.tile([128, 512], F32, tag=f"pp{j}", name=f"pp{j}")
                    for j in range(4)
                ]
                pair_psums[pair].append(psums)
                for kc in range(KC):
                    xt = xtp.tile([128, 1024], BF16, tag="xt", name="xt")
                    rings[(mc * KC + kc) % len(rings)].dma_start(
                        out=xt,
                        in_=xT[kc * 128 : (kc + 1) * 128,
                               mc * 1024 : (mc + 1) * 1024],
                    )
                    for gi in range(2):
                        for j in range(2):
                            nc.tensor.matmul(
                                psums[gi * 2 + j],
                                lhsT=wdqT[:, kc, (g0 + gi) * 128 : (g0 + gi + 1) * 128],
                                rhs=xt[:, j * 512 : (j + 1) * 512],
                                start=(kc == 0),
                                stop=(kc == KC - 1),
                            )
                if inline_drain:
                    emit_chunk_drain(g0, mc, psums)

        def emit_chunk_drain(g0, mc, psums):
            # DVE is idle during the tail; keep ScalarE free for ring-3
            # xt DMA issues.
            for gi in range(2):
                for j in range(2):
                    ysb = ytp.tile([128, 512], F32, tag="ysb", name="ysb")
                    nc.vector.tensor_scalar(
                        out=ysb, in0=psums[gi * 2 + j],
                        scalar1=bias_sb[g0 + gi], scalar2=None, op0=A.add,
                    )
                    g, mg = g0 + gi, mc * 2 + j
                    nc.sync.dma_start(
                        out=yT[g * 128 : (g + 1) * 128, mg * 512 : (mg + 1) * 512],
                        in_=ysb,
                    )

        def emit_pair_tail(pair):
            # bias-add drain on the DVE: emitted after all quant, so it
            # executes during the tail matmul window where DVE is idle -
            # and the scheduler cannot hoist it into the Scalar engine's
            # quant-critical Au/Ac chain.
            g0 = 2 * pair
            for mc, psums in enumerate(pair_psums[pair]):
                for gi in range(2):
                    for j in range(2):
                        ysb = ytp.tile([128, 512], F32, tag="ysb", name="ysb")
                        nc.vector.tensor_scalar(
                            out=ysb, in0=psums[gi * 2 + j],
                            scalar1=bias_sb[g0 + gi], scalar2=None, op0=A.add,
                        )
                        g, mg = g0 + gi, mc * 2 + j
                        nc.sync.dma_start(
                            out=yT[g * 128 : (g + 1) * 128, mg * 512 : (mg + 1) * 512],
                            in_=ysb,
                        )

        cur[("wt", 0)] = first_wt
        emit_prep_rest(0)
        emit_quant_group(0)
        emit_quant_group(1)
        emit_pair_mms(0, [nc.sync, nc.gpsimd])
        emit_quant_group(2)
        emit_quant_group(3)
        emit_pair_tail(0)
        emit_pair_mms(1, [nc.sync, nc.gpsimd, nc.scalar], inline_drain=True)

    nc.compile()
    return nc


_NC_CACHE = None


def _in_maps(x, weight, bias):
    import ml_dtypes

    x = np.ascontiguousarray(x, dtype=np.float32)
    weight = np.ascontiguousarray(weight, dtype=np.float32)
    bias = np.ascontiguousarray(bias, dtype=np.float32)
    xT = np.ascontiguousarray(x.T).astype(ml_dtypes.bfloat16)
    in_maps = []
    for c in range(NCORES):
        in_maps.append(
            {
                "xT": xT,
                "w": weight[c * NSH : (c + 1) * NSH],
                "bias": bias[c * NSH : (c + 1) * NSH].reshape(NSH, 1),
            }
        )
    return in_maps


def kernel(x: np.ndarray, weight: np.ndarray, bias: np.ndarray) -> np.ndarray:
    global _NC_CACHE
    if _NC_CACHE is None:
        _NC_CACHE = build_nc()
    nc = _NC_CACHE
    res = run_bass_kernel_spmd(nc, _in_maps(x, weight, bias), list(range(NCORES)))
    yT = np.concatenate([res.results[c]["yT"] for c in range(NCORES)], axis=0)
    return np.ascontiguousarray(yT.T)


def profile_once(x, weight, bias):
    global _NC_CACHE
    if _NC_CACHE is None:
        _NC_CACHE = build_nc()
    nc = _NC_CACHE
    res = run_bass_kernel_spmd(
        nc, _in_maps(x, weight, bias), list(range(NCORES)),
        trace=True, tmpdir="/tmp/nvfp4_trace",
    )
    print("exec_time_ns:", res.exec_time_ns, "mean:", res.mean_exec_time_ns,
          "max_core:", res.max_exec_time_core_id)
    return res.exec_time_ns


# revision 27
# speedup vs baseline: 1.5222x; 1.0013x over previous
"""NVFP4 fake-quantized linear layer on 8 Trainium2 NeuronCores.

Computes: y = x @ dequant(nvfp4_quantize(weight)).T + bias
  x [8192, 4096] f32, weight [4096, 4096] f32, bias [4096] f32.

Strategy (tensor-parallel, row-wise weight sharding, 512 rows/core):
  - x is transposed and cast to bf16 on the host (layout/precision prep);
    every core receives the full xT [K, M] so the matmul phase streams
    natural [128k, m] tiles with plain DMA - no transpose engine, no
    AllGather on the critical path.
  - Quantization (per-(row, 32-block) MSE scale search, bit-faithful fp32
    incl. fp8-e4m3 scale rounding emulated in fp32) is software-pipelined
    across engines with zero-stall buffer rotation:
      ScalarE (ratio i+1): Au = Relu(12 - c*a2s); Ac = 12 - Au  (clip)
      DVE     (ratio i):   tta = Ac & EXP_MASK (exp2 floor bits)
                           msv = max(tta*MAGIC/2, MAGIC)  (fused clamp:
                                 max(2^e,2)*M/2 == max(2^e*M/2, M))
                           r = Ac + msv ; q = r - msv  (magic RNE round)
                           d = a2s*c - q  (unclipped error -> wt buffer)
      ScalarE: dsq = Square(ratio*d)  (ratio^2 MSE weighting folded in)
      DVE:     e = per-32-block reduce(dsq); argmin bookkeeping
    TensorE transposes the dequantized weights into wdqT bf16.
  - Each group's prep (w load, bmax, b2s, |.|, first clip) is emitted
    under the previous group's final pass, removing the group-boundary
    serial chain from the critical path.
  - Matmul runs over GROUP-PAIRS so each xt tile feeds 4 MMs (2 groups x
    2 m-groups), halving DMA bytes per matmul: pair {0,1} executes hidden
    under quantization of groups 2,3 (xt streamed on two DMA rings);
    pair {2,3} is the tail with inline per-chunk psum drains. Bias is
    added during the PSUM->SBUF drain (DVE for pair 0 - it lands in the
    tail window and keeps ScalarE's quant chain clean; ScalarE for pair
    1). Each core writes yT [512, 8192]; the host concatenates+transposes.
"""

import sys

sys.path.insert(0, "/opt/trn_rl_repo")

from contextlib import ExitStack

import numpy as np

import concourse.bass as bass
import concourse.bacc as bacc
import concourse.tile as tile
from concourse import mybir
from concourse.bass_utils import run_bass_kernel_spmd

A = mybir.AluOpType
AF = mybir.ActivationFunctionType
F32 = mybir.dt.float32
BF16 = mybir.dt.bfloat16
I32 = mybir.dt.int32

NCORES = 8
M, K, N = 8192, 4096, 4096
NSH = N // NCORES          # 512 weight rows per core
NG = NSH // 128            # 4 row groups per core
KC = K // 128              # 32 contraction chunks
KB = K // 32               # 128 blocks per weight row
MG = M // 512              # 16 output m-groups
MCH = 4                    # m-groups per psum chunk (4 banks live)

RATIOS = [float(r) for r in np.linspace(0.7, 1.0, 10)]
MAGIC = 12582912.0         # 1.5 * 2**23 : RNE-to-integer magic constant
INF = float("inf")
EXP_MASK = 0x7F800000      # fp32 exponent field mask
ABS_MASK = 0x7FFFFFFF      # clears the sign bit
TWO_BITS = 0x40000000      # bits of 2.0f; int max == float max for positives
# fp8 e4m3 rounding grid: step = max(2^-9, exp2floor(x) * 2^-3)
MAGIC8_HI = MAGIC / 8.0
MAGIC8_LO = MAGIC / 512.0


def build_nc() -> bass.Bass:
    nc = bacc.Bacc("TRN2", num_devices=NCORES)

    xT = nc.declare_dram_parameter("xT", [K, M], BF16, isOutput=False)
    w = nc.declare_dram_parameter("w", [NSH, K], F32, isOutput=False)
    bias = nc.declare_dram_parameter("bias", [NSH, 1], F32, isOutput=False)
    yT = nc.declare_dram_parameter("yT", [NSH, M], F32, isOutput=True)

    with tile.TileContext(nc) as tc, ExitStack() as ctx:
        big = ctx.enter_context(tc.tile_pool(name="big", bufs=1))
        sm = ctx.enter_context(tc.tile_pool(name="small", bufs=1))
        wtp = ctx.enter_context(tc.tile_pool(name="wtp", bufs=1))
        psum = ctx.enter_context(tc.tile_pool(name="psum", bufs=1, space="PSUM"))
        xtp = ctx.enter_context(tc.tile_pool(name="xtp", bufs=8))
        ytp = ctx.enter_context(tc.tile_pool(name="ytp", bufs=2))

        # persistent w_dq^T, bf16 [128 k-partitions, 32 k-chunks, 512 n]
        wdqT = big.tile([128, KC, NSH], BF16, tag="wdqT", name="wdqT")

        ident = sm.tile([128, 128], BF16, tag="ident", name="ident")
        from concourse.masks import make_identity

        make_identity(nc, ident)

        first_wt = wtp.tile([128, K], F32, tag="wt", name="wt")
        nc.scalar.dma_start(out=first_wt[:, : K // 2], in_=w[0:128, : K // 2])
        nc.scalar.dma_start(out=first_wt[:, K // 2 :], in_=w[0:128, K // 2 :])

        bias_sb = []
        for g in range(NG):
            bsl = sm.tile([128, 1], F32, tag=f"bias{g}", name=f"bias{g}")
            nc.scalar.dma_start(out=bsl, in_=bias[g * 128 : (g + 1) * 128, :])
            bias_sb.append(bsl)

        twelve = sm.tile([128, 1], F32, tag="twelve", name="twelve")
        nc.vector.memset(twelve, 12.0)

        # hoisted per-ratio constant tiles for the argmin bookkeeping
        cconst, rconst = [], []
        for i, ratio in enumerate(RATIOS):
            ct = sm.tile([128, KB], F32, tag=f"cc{i}", name=f"cc{i}")
            nc.vector.memset(ct, float(np.float32(1.0) / np.float32(ratio)))
            cconst.append(ct)
            rt = sm.tile([128, KB], F32, tag=f"rc{i}", name=f"rc{i}")
            nc.vector.memset(rt, float(np.float32(ratio)))
            rconst.append(rt)

        # Per-group quant state; bmax alternates tags so group g's sf can
        # still read it after group g+1's prep has started.
        cur = {}

        def emit_wt_load(g):
            # wt shares its buffer with the ratio loop's d tile; emitting
            # the load right after the last d keeps WAR order correct and
            # gives the DMA the whole final pass to complete.
            wt = wtp.tile([128, K], F32, tag="wt", name="wt")
            nc.scalar.dma_start(out=wt, in_=w[g * 128 : (g + 1) * 128, :])
            cur[("wt", g)] = wt

        def emit_prep_rest(g):
            # bmax/inv/b2s/a2s for group g - emitted under group g-1's
            # final pass so the scalar Abs->Au->Ac chain is off the
            # group-boundary critical path.
            wt = cur.pop(("wt", g))
            wt3 = wt.rearrange("p (b e) -> p b e", e=32)
            bmax = sm.tile([128, KB], F32, tag=f"bmax{g % 2}", name="bmax")
            # two half-width reduces: the first starts as soon as the first
            # half of wt has landed (the g=0 load is split in two DMAs)
            nc.vector.tensor_reduce(
                out=bmax[:, : KB // 2], in_=wt3[:, : KB // 2], axis=mybir.AxisListType.X,
                op=A.max, apply_absolute_value=True,
            )
            nc.vector.tensor_reduce(
                out=bmax[:, KB // 2 :], in_=wt3[:, KB // 2 :], axis=mybir.AxisListType.X,
                op=A.max, apply_absolute_value=True,
            )
            nc.vector.tensor_scalar(out=bmax, in0=bmax, scalar1=1e-12, scalar2=None, op0=A.max)
            inv = sm.tile([128, KB], F32, tag="inv", name="inv")
            nc.vector.reciprocal(out=inv, in_=bmax)
            b2s = big.tile([128, K], F32, tag="b2s", name="b2s")
            b2s3 = b2s.rearrange("p (b e) -> p b e", e=32)
            inv_b = inv.unsqueeze(2).broadcast_to([128, KB, 32])
            nc.vector.scalar_tensor_tensor(
                out=b2s3, in0=wt3, scalar=12.0, in1=inv_b, op0=A.mult, op1=A.mult,
            )
            a2s = big.tile([128, K], F32, tag="a2s", name="a2s")
            nc.scalar.activation(out=a2s, in_=b2s, func=AF.Abs)
            cur["bmax"], cur["b2s"], cur["a2s"] = bmax, b2s, a2s
            # pre-warm ratio 0's clip on ScalarE
            cur["Ac0"] = emit_clip(0)

        def emit_clip(i):
            # Ac = min(a2s*c_i, 12) via 12 - Relu(12 - c_i*a2s) on ScalarE
            c = float(np.float32(1.0) / np.float32(RATIOS[i]))
            Au = big.tile([128, K], F32, tag="sE", name="Au")
            nc.scalar.activation(
                out=Au, in_=cur["a2s"], func=AF.Relu, scale=-c, bias=twelve,
            )
            Ac = big.tile([128, K], F32, tag="sA", name="Ac")
            nc.scalar.activation(
                out=Ac, in_=Au, func=AF.Identity, scale=-1.0, bias=twelve,
            )
            return Ac

        def emit_mask(Ac):
            # tta = exp2floor-bits(Ac);  msv = max(tta*MAGIC/2, MAGIC)
            # (identity: max(2^e, 2)*M/2 == max(2^e*M/2, M))
            tta = big.tile([128, K], F32, tag="sB", name="tta")
            nc.vector.tensor_scalar(
                out=tta.bitcast(I32), in0=Ac.bitcast(I32),
                scalar1=EXP_MASK, scalar2=None, op0=A.bitwise_and,
            )
            msv = big.tile([128, K], F32, tag="sD", name="msv")
            nc.vector.tensor_scalar(
                out=msv, in0=tta, scalar1=MAGIC / 2.0, scalar2=MAGIC,
                op0=A.mult, op1=A.max,
            )
            return msv

        def emit_search(g):
            a2s = cur["a2s"]
            best_e = sm.tile([128, KB], F32, tag="best_e", name="best_e")
            nc.vector.memset(best_e, INF)
            best_c = sm.tile([128, KB], F32, tag="best_c", name="best_c")
            nc.vector.memset(best_c, 0.0)
            best_r = sm.tile([128, KB], F32, tag="best_r", name="best_r")
            nc.vector.memset(best_r, 1.0)
            cur["best_c"], cur["best_r"] = best_c, best_r

            # Software-pipelined MSE search: ScalarE computes ratio i+1's
            # clipped operand while DVE rounds ratio i; DVE hides the
            # Square latency under ratio i+1's tta/msv. Zero-stall.
            Ac = cur.pop("Ac0")
            msv = emit_mask(Ac)
            for i, ratio in enumerate(RATIOS):
                c = float(np.float32(1.0) / np.float32(ratio))
                # r = Ac + msv ; q = r - msv  (RNE onto the e2m1 grid)
                r_ = big.tile([128, K], F32, tag="sC", name="r")
                nc.vector.tensor_tensor(out=r_, in0=Ac, in1=msv, op=A.add)
                q_ = big.tile([128, K], F32, tag="sB", name="q")
                nc.vector.scalar_tensor_tensor(
                    out=q_, in0=msv, scalar=-1.0, in1=r_, op0=A.mult, op1=A.add,
                )
                if i + 1 < len(RATIOS):
                    Ac = emit_clip(i + 1)
                # d = a2s*c - q (unclipped error, matches reference MSE);
                # lives in the idle wt buffer
                d_ = wtp.tile([128, K], F32, tag="wt", name="d")
                nc.vector.scalar_tensor_tensor(
                    out=d_, in0=a2s, scalar=c, in1=q_, op0=A.mult, op1=A.subtract,
                )
                if i + 1 < len(RATIOS):
                    msv = emit_mask(Ac)
                # dsq = (ratio*d)^2 : folds the ratio^2 MSE weighting in
                dsq = big.tile([128, K], F32, tag="sC", name="dsq")
                nc.scalar.activation(
                    out=dsq, in_=d_, func=AF.Square, scale=float(np.float32(ratio)),
                )
                e_ = sm.tile([128, KB], F32, tag="e", name="e")
                nc.vector.tensor_reduce(
                    out=e_, in_=dsq.rearrange("p (b e) -> p b e", e=32),
                    axis=mybir.AxisListType.X, op=A.add,
                )
                mask = sm.tile([128, KB], I32, tag="mask", name="mask")
                nc.vector.tensor_tensor(out=mask, in0=e_, in1=best_e, op=A.is_lt)
                nc.vector.tensor_tensor(out=best_e, in0=e_, in1=best_e, op=A.min)
                nc.vector.copy_predicated(out=best_c, mask=mask, data=cconst[i])
                nc.vector.copy_predicated(out=best_r, mask=mask, data=rconst[i])

        def emit_final(g, prefetch_next):
            bmax, b2s = cur["bmax"], cur["b2s"]
            best_c, best_r = cur["best_c"], cur["best_r"]
            # scale factor sf = bmax * best_r / 6, rounded to fp8 e4m3 (RNE,
            # subnormal-aware) emulated in fp32, then halved (q = q2/2).
            sf = sm.tile([128, KB], F32, tag="sf", name="sf")
            nc.vector.scalar_tensor_tensor(
                out=sf, in0=bmax, scalar=1.0 / 6.0, in1=best_r, op0=A.mult, op1=A.mult,
            )
            eb8 = sm.tile([128, KB], F32, tag="eb8", name="eb8")
            nc.vector.tensor_scalar(
                out=eb8.bitcast(I32), in0=sf.bitcast(I32),
                scalar1=EXP_MASK, scalar2=None, op0=A.bitwise_and,
            )
            ms8 = sm.tile([128, KB], F32, tag="ms8", name="ms8")
            nc.vector.tensor_scalar(
                out=ms8, in0=eb8, scalar1=MAGIC8_HI, scalar2=MAGIC8_LO, op0=A.mult, op1=A.max,
            )
            nc.vector.tensor_tensor(out=sf, in0=sf, in1=ms8, op=A.add)
            nc.vector.tensor_tensor(out=sf, in0=sf, in1=ms8, op=A.subtract)
            nc.vector.tensor_scalar(out=sf, in0=sf, scalar1=0.5, scalar2=None, op0=A.mult)

            # final quantization with the chosen scale (signed)
            B2f = big.tile([128, K], F32, tag="sA", name="B2f")
            B2f3 = B2f.rearrange("p (b e) -> p b e", e=32)
            bc_b = best_c.unsqueeze(2).broadcast_to([128, KB, 32])
            nc.vector.tensor_tensor(out=B2f3, in0=b2s.rearrange("p (b e) -> p b e", e=32), in1=bc_b, op=A.mult)
            if prefetch_next:
                # next group's bmax/b2s/a2s + scalar clip chain, hidden
                # under this group's final pass
                emit_prep_rest(g + 1)
            ttaf = big.tile([128, K], F32, tag="sB", name="ttaf")
            nc.vector.tensor_scalar(
                out=ttaf.bitcast(I32), in0=B2f.bitcast(I32),
                scalar1=EXP_MASK, scalar2=None, op0=A.bitwise_and,
            )
            msvf = big.tile([128, K], F32, tag="sD", name="msvf")
            nc.vector.tensor_scalar(
                out=msvf, in0=ttaf, scalar1=MAGIC / 2.0, scalar2=MAGIC,
                op0=A.mult, op1=A.max,
            )
            rf = big.tile([128, K], F32, tag="sC", name="rf")
            nc.vector.tensor_tensor(out=rf, in0=B2f, in1=msvf, op=A.add)
            qf = big.tile([128, K], F32, tag="sE", name="qf")
            nc.vector.scalar_tensor_tensor(
                out=qf, in0=msvf, scalar=-1.0, in1=rf, op0=A.mult, op1=A.add,
            )
            qc = big.tile([128, K], F32, tag="sB", name="qc")
            nc.vector.tensor_scalar(
                out=qc, in0=qf, scalar1=12.0, scalar2=-12.0, op0=A.min, op1=A.max,
            )
            wdq = big.tile([128, K], BF16, tag="wdq", name="wdq")
            sf_b = sf.unsqueeze(2).broadcast_to([128, KB, 32])
            nc.vector.tensor_tensor(
                out=wdq.rearrange("p (b e) -> p b e", e=32),
                in0=qc.rearrange("p (b e) -> p b e", e=32),
                in1=sf_b, op=A.mult,
            )

            # transpose into wdqT[:, kc, g*128:(g+1)*128]
            for kc in range(KC):
                pt = psum.tile([128, 128], BF16, tag="ptr", bufs=2, name="pt")
                nc.tensor.transpose(pt, wdq[:, kc * 128 : (kc + 1) * 128], ident)
                nc.scalar.copy(out=wdqT[:, kc, g * 128 : (g + 1) * 128], in_=pt)

        def emit_quant_group(g):
            emit_search(g)
            if g + 1 < NG:
                emit_wt_load(g + 1)
            emit_final(g, prefetch_next=(g + 1 < NG))

        # Matmul runs over GROUP-PAIRS: each xt tile feeds 4 MMs (2 groups
        # x 2 m-groups), halving DMA bytes per MM vs per-group passes.
        # Pair {0,1} hides under quant of groups 2,3; pair {2,3} is the tail.
        pair_psums = {}

        def emit_pair_mms(pair, rings, inline_drain=False):
            g0 = 2 * pair
            pair_psums[pair] = []
            for mc in range(M // 1024):
                psums = [
                    psum.tile([128, 512], F32, tag=f"pp{j}", name=f"pp{j}")
                    for j in range(4)
                ]
                pair_psums[pair].append(psums)
                for kc in range(KC):
                    xt = xtp.tile([128, 1024], BF16, tag="xt", name="xt")
                    rings[(mc * KC + kc) % len(rings)].dma_start(
                        out=xt,
                        in_=xT[kc * 128 : (kc + 1) * 128,
                               mc * 1024 : (mc + 1) * 1024],
                    )
                    for gi in range(2):
                        for j in range(2):
                            nc.tensor.matmul(
                                psums[gi * 2 + j],
                                lhsT=wdqT[:, kc, (g0 + gi) * 128 : (g0 + gi + 1) * 128],
                                rhs=xt[:, j * 512 : (j + 1) * 512],
                                start=(kc == 0),
                                stop=(kc == KC - 1),
                            )
                if inline_drain:
                    emit_chunk_drain(g0, mc, psums)

        def emit_chunk_drain(g0, mc, psums):
            # DVE is idle during the tail; keep ScalarE free for ring-3
            # xt DMA issues.
            for gi in range(2):
                for j in range(2):
                    ysb = ytp.tile([128, 512], F32, tag="ysb", name="ysb")
                    nc.vector.tensor_scalar(
                        out=ysb, in0=psums[gi * 2 + j],
                        scalar1=bias_sb[g0 + gi], scalar2=None, op0=A.add,
                    )
                    g, mg = g0 + gi, mc * 2 + j
                    nc.sync.dma_start(
                        out=yT[g * 128 : (g + 1) * 128, mg * 512 : (mg + 1) * 512],
                        in_=ysb,
                    )

        def emit_pair_tail(pair):
            # bias-add drain on the DVE: emitted after all quant, so it
            # executes during the tail matmul window where DVE is idle -
            # and the scheduler cannot hoist it into the Scalar engine's
            # quant-critical Au/Ac chain.
            g0 = 2 * pair
            for mc, psums in enumerate(pair_psums[pair]):
                for gi in range(2):
                    for j in range(2):
                        ysb = ytp.tile([128, 512], F32, tag="ysb", name="ysb")
                        nc.vector.tensor_scalar(
                            out=ysb, in0=psums[gi * 2 + j],
                            scalar1=bias_sb[g0 + gi], scalar2=None, op0=A.add,
                        )
                        g, mg = g0 + gi, mc * 2 + j
                        nc.sync.dma_start(
                            out=yT[g * 128 : (g + 1) * 128, mg * 512 : (mg + 1) * 512],
                            in_=ysb,
                        )

        cur[("wt", 0)] = first_wt
        emit_prep_rest(0)
        emit_quant_group(0)
        emit_quant_group(1)
        emit_pair_mms(0, [nc.sync, nc.gpsimd])
        emit_quant_group(2)
        emit_quant_group(3)
        emit_pair_tail(0)
        emit_pair_mms(1, [nc.sync, nc.gpsimd, nc.scalar], inline_drain=True)

    nc.compile()
    return nc


_NC_CACHE = None


def _in_maps(x, weight, bias):
    import ml_dtypes

    x = np.ascontiguousarray(x, dtype=np.float32)
    weight = np.ascontiguousarray(weight, dtype=np.float32)
    bias = np.ascontiguousarray(bias, dtype=np.float32)
    xT = np.ascontiguousarray(x.T).astype(ml_dtypes.bfloat16)
    in_maps = []
    for c in range(NCORES):
        in_maps.append(
            {
                "xT": xT,
                "w": weight[c * NSH : (c + 1) * NSH],
                "bias": bias[c * NSH : (c + 1) * NSH].reshape(NSH, 1),
            }
        )
    return in_maps


def kernel(x: np.ndarray, weight: np.ndarray, bias: np.ndarray) -> np.ndarray:
    global _NC_CACHE
    if _NC_CACHE is None:
        _NC_CACHE = build_nc()
    nc = _NC_CACHE
    res = run_bass_kernel_spmd(nc, _in_maps(x, weight, bias), list(range(NCORES)))
    yT = np.concatenate([res.results[c]["yT"] for c in range(NCORES)], axis=0)
    return np.ascontiguousarray(yT.T)


def profile_once(x, weight, bias):
    global _NC_CACHE
    if _NC_CACHE is None:
        _NC_CACHE = build_nc()
    nc = _NC_CACHE
    res = run_bass_kernel_spmd(
        nc, _in_maps(x, weight, bias), list(range(NCORES)),
        trace=True, tmpdir="/tmp/nvfp4_trace",
    )
    print("exec_time_ns:", res.exec_time_ns, "mean:", res.mean_exec_time_ns,
          "max_core:", res.max_exec_time_core_id)
    return res.exec_time_ns


# revision 28
# speedup vs baseline: 1.5261x; 1.0025x over previous
"""NVFP4 fake-quantized linear layer on 8 Trainium2 NeuronCores.

Computes: y = x @ dequant(nvfp4_quantize(weight)).T + bias
  x [8192, 4096] f32, weight [4096, 4096] f32, bias [4096] f32.

Strategy (tensor-parallel, row-wise weight sharding, 512 rows/core):
  - x is transposed and cast to bf16 on the host (layout/precision prep);
    every core receives the full xT [K, M] so the matmul phase streams
    natural [128k, m] tiles with plain DMA - no transpose engine, no
    AllGather on the critical path.
  - Quantization (per-(row, 32-block) MSE scale search, bit-faithful fp32
    incl. fp8-e4m3 scale rounding emulated in fp32) is software-pipelined
    across engines with zero-stall buffer rotation:
      ScalarE (ratio i+1): Au = Relu(12 - c*a2s); Ac = 12 - Au  (clip)
      DVE     (ratio i):   tta = Ac & EXP_MASK (exp2 floor bits)
                           msv = max(tta*MAGIC/2, MAGIC)  (fused clamp:
                                 max(2^e,2)*M/2 == max(2^e*M/2, M))
                           r = Ac + msv ; q = r - msv  (magic RNE round)
                           d = a2s*c - q  (unclipped error -> wt buffer)
      ScalarE: dsq = Square(ratio*d)  (ratio^2 MSE weighting folded in)
      DVE:     e = per-32-block reduce(dsq); argmin bookkeeping
    TensorE transposes the dequantized weights into wdqT bf16.
  - Each group's prep (w load, bmax, b2s, |.|, first clip) is emitted
    under the previous group's final pass, removing the group-boundary
    serial chain from the critical path.
  - Matmul runs over GROUP-PAIRS so each xt tile feeds 4 MMs (2 groups x
    2 m-groups), halving DMA bytes per matmul: pair {0,1} executes hidden
    under quantization of groups 2,3 (xt streamed on two DMA rings);
    pair {2,3} is the tail with inline per-chunk psum drains. Bias is
    added during the PSUM->SBUF drain (DVE for pair 0 - it lands in the
    tail window and keeps ScalarE's quant chain clean; ScalarE for pair
    1). Each core writes yT [512, 8192]; the host concatenates+transposes.
"""

import sys

sys.path.insert(0, "/opt/trn_rl_repo")

from contextlib import ExitStack

import numpy as np

import concourse.bass as bass
import concourse.bacc as bacc
import concourse.tile as tile
from concourse import mybir
from concourse.bass_utils import run_bass_kernel_spmd

A = mybir.AluOpType
AF = mybir.ActivationFunctionType
F32 = mybir.dt.float32
BF16 = mybir.dt.bfloat16
I32 = mybir.dt.int32

NCORES = 8
M, K, N = 8192, 4096, 4096
NSH = N // NCORES          # 512 weight rows per core
NG = NSH // 128            # 4 row groups per core
KC = K // 128              # 32 contraction chunks
KB = K // 32               # 128 blocks per weight row
MG = M // 512              # 16 output m-groups
MCH = 4                    # m-groups per psum chunk (4 banks live)

RATIOS = [float(r) for r in np.linspace(0.7, 1.0, 10)]
MAGIC = 12582912.0         # 1.5 * 2**23 : RNE-to-integer magic constant
INF = float("inf")
EXP_MASK = 0x7F800000      # fp32 exponent field mask
ABS_MASK = 0x7FFFFFFF      # clears the sign bit
TWO_BITS = 0x40000000      # bits of 2.0f; int max == float max for positives
# fp8 e4m3 rounding grid: step = max(2^-9, exp2floor(x) * 2^-3)
MAGIC8_HI = MAGIC / 8.0
MAGIC8_LO = MAGIC / 512.0


def build_nc() -> bass.Bass:
    nc = bacc.Bacc("TRN2", num_devices=NCORES)

    xT = nc.declare_dram_parameter("xT", [K, M], BF16, isOutput=False)
    w = nc.declare_dram_parameter("w", [NSH, K], F32, isOutput=False)
    bias = nc.declare_dram_parameter("bias", [NSH, 1], F32, isOutput=False)
    yT = nc.declare_dram_parameter("yT", [NSH, M], F32, isOutput=True)

    with tile.TileContext(nc) as tc, ExitStack() as ctx:
        big = ctx.enter_context(tc.tile_pool(name="big", bufs=1))
        sm = ctx.enter_context(tc.tile_pool(name="small", bufs=1))
        wtp = ctx.enter_context(tc.tile_pool(name="wtp", bufs=1))
        psum = ctx.enter_context(tc.tile_pool(name="psum", bufs=1, space="PSUM"))
        xtp = ctx.enter_context(tc.tile_pool(name="xtp", bufs=8))
        ytp = ctx.enter_context(tc.tile_pool(name="ytp", bufs=2))

        # persistent w_dq^T, bf16 [128 k-partitions, 32 k-chunks, 512 n]
        wdqT = big.tile([128, KC, NSH], BF16, tag="wdqT", name="wdqT")

        ident = sm.tile([128, 128], BF16, tag="ident", name="ident")
        from concourse.masks import make_identity

        make_identity(nc, ident)

        first_wt = wtp.tile([128, K], F32, tag="wt", name="wt")
        nc.scalar.dma_start(out=first_wt[:, : K // 2], in_=w[0:128, : K // 2])
        nc.scalar.dma_start(out=first_wt[:, K // 2 :], in_=w[0:128, K // 2 :])

        bias_sb = []
        for g in range(NG):
            bsl = sm.tile([128, 1], F32, tag=f"bias{g}", name=f"bias{g}")
            nc.scalar.dma_start(out=bsl, in_=bias[g * 128 : (g + 1) * 128, :])
            bias_sb.append(bsl)

        twelve = sm.tile([128, 1], F32, tag="twelve", name="twelve")
        nc.vector.memset(twelve, 12.0)

        # hoisted per-ratio constant tiles for the argmin bookkeeping
        cconst, rconst = [], []
        for i, ratio in enumerate(RATIOS):
            ct = sm.tile([128, KB], F32, tag=f"cc{i}", name=f"cc{i}")
            nc.vector.memset(ct, float(np.float32(1.0) / np.float32(ratio)))
            cconst.append(ct)
            rt = sm.tile([128, KB], F32, tag=f"rc{i}", name=f"rc{i}")
            nc.vector.memset(rt, float(np.float32(ratio)))
            rconst.append(rt)

        # Per-group quant state; bmax alternates tags so group g's sf can
        # still read it after group g+1's prep has started.
        cur = {}

        def emit_wt_load(g):
            # wt shares its buffer with the ratio loop's d tile; emitting
            # the load right after the last d keeps WAR order correct and
            # gives the DMA the whole final pass to complete.
            wt = wtp.tile([128, K], F32, tag="wt", name="wt")
            nc.scalar.dma_start(out=wt, in_=w[g * 128 : (g + 1) * 128, :])
            cur[("wt", g)] = wt

        def emit_prep_rest(g):
            # bmax/inv/b2s/a2s for group g - emitted under group g-1's
            # final pass so the scalar Abs->Au->Ac chain is off the
            # group-boundary critical path.
            wt = cur.pop(("wt", g))
            wt3 = wt.rearrange("p (b e) -> p b e", e=32)
            bmax = sm.tile([128, KB], F32, tag=f"bmax{g % 2}", name="bmax")
            # two half-width reduces: the first starts as soon as the first
            # half of wt has landed (the g=0 load is split in two DMAs)
            nc.vector.tensor_reduce(
                out=bmax[:, : KB // 2], in_=wt3[:, : KB // 2], axis=mybir.AxisListType.X,
                op=A.max, apply_absolute_value=True,
            )
            nc.vector.tensor_reduce(
                out=bmax[:, KB // 2 :], in_=wt3[:, KB // 2 :], axis=mybir.AxisListType.X,
                op=A.max, apply_absolute_value=True,
            )
            nc.vector.tensor_scalar(out=bmax, in0=bmax, scalar1=1e-12, scalar2=None, op0=A.max)
            inv = sm.tile([128, KB], F32, tag="inv", name="inv")
            nc.vector.reciprocal(out=inv, in_=bmax)
            b2s = big.tile([128, K], F32, tag="b2s", name="b2s")
            b2s3 = b2s.rearrange("p (b e) -> p b e", e=32)
            inv_b = inv.unsqueeze(2).broadcast_to([128, KB, 32])
            nc.vector.scalar_tensor_tensor(
                out=b2s3, in0=wt3, scalar=12.0, in1=inv_b, op0=A.mult, op1=A.mult,
            )
            a2s = big.tile([128, K], F32, tag="a2s", name="a2s")
            nc.scalar.activation(out=a2s, in_=b2s, func=AF.Abs)
            cur["bmax"], cur["b2s"], cur["a2s"] = bmax, b2s, a2s
            if g == 0:
                # startup: one DVE TS beats waiting on the ScalarE
                # Abs->Relu->Identity chain
                c0 = float(np.float32(1.0) / np.float32(RATIOS[0]))
                Ac0 = big.tile([128, K], F32, tag="sA", name="Ac0")
                nc.vector.tensor_scalar(
                    out=Ac0, in0=a2s, scalar1=c0, scalar2=12.0, op0=A.mult, op1=A.min,
                )
                cur["Ac0"] = Ac0
            else:
                # pre-warm ratio 0's clip on ScalarE
                cur["Ac0"] = emit_clip(0)

        def emit_clip(i):
            # Ac = min(a2s*c_i, 12) via 12 - Relu(12 - c_i*a2s) on ScalarE
            c = float(np.float32(1.0) / np.float32(RATIOS[i]))
            Au = big.tile([128, K], F32, tag="sE", name="Au")
            nc.scalar.activation(
                out=Au, in_=cur["a2s"], func=AF.Relu, scale=-c, bias=twelve,
            )
            Ac = big.tile([128, K], F32, tag="sA", name="Ac")
            nc.scalar.activation(
                out=Ac, in_=Au, func=AF.Identity, scale=-1.0, bias=twelve,
            )
            return Ac

        def emit_mask(Ac):
            # tta = exp2floor-bits(Ac);  msv = max(tta*MAGIC/2, MAGIC)
            # (identity: max(2^e, 2)*M/2 == max(2^e*M/2, M))
            tta = big.tile([128, K], F32, tag="sB", name="tta")
            nc.vector.tensor_scalar(
                out=tta.bitcast(I32), in0=Ac.bitcast(I32),
                scalar1=EXP_MASK, scalar2=None, op0=A.bitwise_and,
            )
            msv = big.tile([128, K], F32, tag="sD", name="msv")
            nc.vector.tensor_scalar(
                out=msv, in0=tta, scalar1=MAGIC / 2.0, scalar2=MAGIC,
                op0=A.mult, op1=A.max,
            )
            return msv

        def emit_search(g):
            a2s = cur["a2s"]
            best_e = sm.tile([128, KB], F32, tag="best_e", name="best_e")
            nc.vector.memset(best_e, INF)
            best_c = sm.tile([128, KB], F32, tag="best_c", name="best_c")
            nc.vector.memset(best_c, 0.0)
            best_r = sm.tile([128, KB], F32, tag="best_r", name="best_r")
            nc.vector.memset(best_r, 1.0)
            cur["best_c"], cur["best_r"] = best_c, best_r

            # Software-pipelined MSE search: ScalarE computes ratio i+1's
            # clipped operand while DVE rounds ratio i; DVE hides the
            # Square latency under ratio i+1's tta/msv. Zero-stall.
            Ac = cur.pop("Ac0")
            msv = emit_mask(Ac)
            for i, ratio in enumerate(RATIOS):
                c = float(np.float32(1.0) / np.float32(ratio))
                # r = Ac + msv ; q = r - msv  (RNE onto the e2m1 grid)
                r_ = big.tile([128, K], F32, tag="sC", name="r")
                nc.vector.tensor_tensor(out=r_, in0=Ac, in1=msv, op=A.add)
                q_ = big.tile([128, K], F32, tag="sB", name="q")
                nc.vector.scalar_tensor_tensor(
                    out=q_, in0=msv, scalar=-1.0, in1=r_, op0=A.mult, op1=A.add,
                )
                if i + 1 < len(RATIOS):
                    Ac = emit_clip(i + 1)
                # d = a2s*c - q (unclipped error, matches reference MSE);
                # lives in the idle wt buffer
                d_ = wtp.tile([128, K], F32, tag="wt", name="d")
                nc.vector.scalar_tensor_tensor(
                    out=d_, in0=a2s, scalar=c, in1=q_, op0=A.mult, op1=A.subtract,
                )
                if i + 1 < len(RATIOS):
                    msv = emit_mask(Ac)
                # dsq = (ratio*d)^2 : folds the ratio^2 MSE weighting in
                dsq = big.tile([128, K], F32, tag="sC", name="dsq")
                nc.scalar.activation(
                    out=dsq, in_=d_, func=AF.Square, scale=float(np.float32(ratio)),
                )
                e_ = sm.tile([128, KB], F32, tag="e", name="e")
                nc.vector.tensor_reduce(
                    out=e_, in_=dsq.rearrange("p (b e) -> p b e", e=32),
                    axis=mybir.AxisListType.X, op=A.add,
                )
                mask = sm.tile([128, KB], I32, tag="mask", name="mask")
                nc.vector.tensor_tensor(out=mask, in0=e_, in1=best_e, op=A.is_lt)
                nc.vector.tensor_tensor(out=best_e, in0=e_, in1=best_e, op=A.min)
                nc.vector.copy_predicated(out=best_c, mask=mask, data=cconst[i])
                nc.vector.copy_predicated(out=best_r, mask=mask, data=rconst[i])

        def emit_final(g, prefetch_next):
            bmax, b2s = cur["bmax"], cur["b2s"]
            best_c, best_r = cur["best_c"], cur["best_r"]
            # scale factor sf = bmax * best_r / 6, rounded to fp8 e4m3 (RNE,
            # subnormal-aware) emulated in fp32, then halved (q = q2/2).
            sf = sm.tile([128, KB], F32, tag="sf", name="sf")
            nc.vector.scalar_tensor_tensor(
                out=sf, in0=bmax, scalar=1.0 / 6.0, in1=best_r, op0=A.mult, op1=A.mult,
            )
            eb8 = sm.tile([128, KB], F32, tag="eb8", name="eb8")
            nc.vector.tensor_scalar(
                out=eb8.bitcast(I32), in0=sf.bitcast(I32),
                scalar1=EXP_MASK, scalar2=None, op0=A.bitwise_and,
            )
            ms8 = sm.tile([128, KB], F32, tag="ms8", name="ms8")
            nc.vector.tensor_scalar(
                out=ms8, in0=eb8, scalar1=MAGIC8_HI, scalar2=MAGIC8_LO, op0=A.mult, op1=A.max,
            )
            nc.vector.tensor_tensor(out=sf, in0=sf, in1=ms8, op=A.add)
            nc.vector.tensor_tensor(out=sf, in0=sf, in1=ms8, op=A.subtract)
            nc.vector.tensor_scalar(out=sf, in0=sf, scalar1=0.5, scalar2=None, op0=A.mult)

            # final quantization with the chosen scale (signed)
            B2f = big.tile([128, K], F32, tag="sA", name="B2f")
            B2f3 = B2f.rearrange("p (b e) -> p b e", e=32)
            bc_b = best_c.unsqueeze(2).broadcast_to([128, KB, 32])
            nc.vector.tensor_tensor(out=B2f3, in0=b2s.rearrange("p (b e) -> p b e", e=32), in1=bc_b, op=A.mult)
            if prefetch_next:
                # next group's bmax/b2s/a2s + scalar clip chain, hidden
                # under this group's final pass
                emit_prep_rest(g + 1)
            ttaf = big.tile([128, K], F32, tag="sB", name="ttaf")
            nc.vector.tensor_scalar(
                out=ttaf.bitcast(I32), in0=B2f.bitcast(I32),
                scalar1=EXP_MASK, scalar2=None, op0=A.bitwise_and,
            )
            msvf = big.tile([128, K], F32, tag="sD", name="msvf")
            nc.vector.tensor_scalar(
                out=msvf, in0=ttaf, scalar1=MAGIC / 2.0, scalar2=MAGIC,
                op0=A.mult, op1=A.max,
            )
            rf = big.tile([128, K], F32, tag="sC", name="rf")
            nc.vector.tensor_tensor(out=rf, in0=B2f, in1=msvf, op=A.add)
            qf = big.tile([128, K], F32, tag="sE", name="qf")
            nc.vector.scalar_tensor_tensor(
                out=qf, in0=msvf, scalar=-1.0, in1=rf, op0=A.mult, op1=A.add,
            )
            qc = big.tile([128, K], F32, tag="sB", name="qc")
            nc.vector.tensor_scalar(
                out=qc, in0=qf, scalar1=12.0, scalar2=-12.0, op0=A.min, op1=A.max,
            )
            wdq = big.tile([128, K], BF16, tag="wdq", name="wdq")
            sf_b = sf.unsqueeze(2).broadcast_to([128, KB, 32])
            nc.vector.tensor_tensor(
                out=wdq.rearrange("p (b e) -> p b e", e=32),
                in0=qc.rearrange("p (b e) -> p b e", e=32),
                in1=sf_b, op=A.mult,
            )

            # transpose into wdqT[:, kc, g*128:(g+1)*128]
            for kc in range(KC):
                pt = psum.tile([128, 128], BF16, tag="ptr", bufs=4, name="pt")
                nc.tensor.transpose(pt, wdq[:, kc * 128 : (kc + 1) * 128], ident)
                nc.scalar.copy(out=wdqT[:, kc, g * 128 : (g + 1) * 128], in_=pt)

        def emit_quant_group(g):
            emit_search(g)
            if g + 1 < NG:
                emit_wt_load(g + 1)
            emit_final(g, prefetch_next=(g + 1 < NG))

        # Matmul runs over GROUP-PAIRS: each xt tile feeds 4 MMs (2 groups
        # x 2 m-groups), halving DMA bytes per MM vs per-group passes.
        # Pair {0,1} hides under quant of groups 2,3; pair {2,3} is the tail.
        pair_psums = {}

        def emit_pair_mms(pair, rings, inline_drain=False):
            g0 = 2 * pair
            pair_psums[pair] = []
            for mc in range(M // 1024):
                psums = [
                    psum.tile([128, 512], F32, tag=f"pp{j}", name=f"pp{j}")
                    for j in range(4)
                ]
                pair_psums[pair].append(psums)
                for kc in range(KC):
                    xt = xtp.tile([128, 1024], BF16, tag="xt", name="xt")
                    rings[(mc * KC + kc) % len(rings)].dma_start(
                        out=xt,
                        in_=xT[kc * 128 : (kc + 1) * 128,
                               mc * 1024 : (mc + 1) * 1024],
                    )
                    for gi in range(2):
                        for j in range(2):
                            nc.tensor.matmul(
                                psums[gi * 2 + j],
                                lhsT=wdqT[:, kc, (g0 + gi) * 128 : (g0 + gi + 1) * 128],
                                rhs=xt[:, j * 512 : (j + 1) * 512],
                                start=(kc == 0),
                                stop=(kc == KC - 1),
                            )
                if inline_drain:
                    emit_chunk_drain(g0, mc, psums)

        def emit_chunk_drain(g0, mc, psums):
            # DVE is idle during the tail; keep ScalarE free for ring-3
            # xt DMA issues.
            for gi in range(2):
                for j in range(2):
                    ysb = ytp.tile([128, 512], F32, tag="ysb", name="ysb")
                    nc.vector.tensor_scalar(
                        out=ysb, in0=psums[gi * 2 + j],
                        scalar1=bias_sb[g0 + gi], scalar2=None, op0=A.add,
                    )
                    g, mg = g0 + gi, mc * 2 + j
                    nc.sync.dma_start(
                        out=yT[g * 128 : (g + 1) * 128, mg * 512 : (mg + 1) * 512],
                        in_=ysb,
                    )

        def emit_pair_tail(pair):
            # bias-add drain on the DVE: emitted after all quant, so it
            # executes during the tail matmul window where DVE is idle -
            # and the scheduler cannot hoist it into the Scalar engine's
            # quant-critical Au/Ac chain.
            g0 = 2 * pair
            for mc, psums in enumerate(pair_psums[pair]):
                for gi in range(2):
                    for j in range(2):
                        ysb = ytp.tile([128, 512], F32, tag="ysb", name="ysb")
                        nc.vector.tensor_scalar(
                            out=ysb, in0=psums[gi * 2 + j],
                            scalar1=bias_sb[g0 + gi], scalar2=None, op0=A.add,
                        )
                        g, mg = g0 + gi, mc * 2 + j
                        nc.sync.dma_start(
                            out=yT[g * 128 : (g + 1) * 128, mg * 512 : (mg + 1) * 512],
                            in_=ysb,
                        )

        cur[("wt", 0)] = first_wt
        emit_prep_rest(0)
        emit_quant_group(0)
        emit_quant_group(1)
        emit_pair_mms(0, [nc.sync, nc.gpsimd])
        emit_quant_group(2)
        emit_quant_group(3)
        emit_pair_tail(0)
        emit_pair_mms(1, [nc.sync, nc.gpsimd, nc.scalar], inline_drain=True)

    nc.compile()
    return nc


_NC_CACHE = None


def _in_maps(x, weight, bias):
    import ml_dtypes

    x = np.ascontiguousarray(x, dtype=np.float32)
    weight = np.ascontiguousarray(weight, dtype=np.float32)
    bias = np.ascontiguousarray(bias, dtype=np.float32)
    xT = np.ascontiguousarray(x.T).astype(ml_dtypes.bfloat16)
    in_maps = []
    for c in range(NCORES):
        in_maps.append(
            {
                "xT": xT,
                "w": weight[c * NSH : (c + 1) * NSH],
                "bias": bias[c * NSH : (c + 1) * NSH].reshape(NSH, 1),
            }
        )
    return in_maps


def kernel(x: np.ndarray, weight: np.ndarray, bias: np.ndarray) -> np.ndarray:
    global _NC_CACHE
    if _NC_CACHE is None:
        _NC_CACHE = build_nc()
    nc = _NC_CACHE
    res = run_bass_kernel_spmd(nc, _in_maps(x, weight, bias), list(range(NCORES)))
    yT = np.concatenate([res.results[c]["yT"] for c in range(NCORES)], axis=0)
    return np.ascontiguousarray(yT.T)


def profile_once(x, weight, bias):
    global _NC_CACHE
    if _NC_CACHE is None:
        _NC_CACHE = build_nc()
    nc = _NC_CACHE
    res = run_bass_kernel_spmd(
        nc, _in_maps(x, weight, bias), list(range(NCORES)),
        trace=True, tmpdir="/tmp/nvfp4_trace",
    )
    print("exec_time_ns:", res.exec_time_ns, "mean:", res.mean_exec_time_ns,
          "max_core:", res.max_exec_time_core_id)
    return res.exec_time_ns
